# revision 2
# baseline (speedup 1.0000x reference)
"""Trainium2 Bass kernel for nn_MemoryRamModule (scatter_memory).

Strategy: the reference is a strictly-sequential 32768-step scan with a
(mem[100,512], h[512]) carry, but the memory decays per step by (1-aw),
aw ~ softmax ~ 1/100, so carry influence dies off as e^(-0.01*B). We split
time into 64 chunks of 512 steps, run 8 independent chunk-scans per core
(batched), each with a burn-in re-deriving the carry. Scan g reads input
rows [g*512-B_burn, g*512+512), zero-padded below row 0 (zero inputs
provably keep the carry exactly zero), and emits its last 512 steps as
output rows [g*512, (g+1)*512).

Per core: phase 1 projects its X slab through all x-side weight columns
(one big matmul -> PX in DRAM); phase 2 runs the 8 scans batched, with the
per-step recurrent work done as small PE matmuls (h-projections, gated
memory read, rank-1 + decay memory update) plus DVE/ACT softmax/gate ops.

All host<->device IO is fp16 (the axon tunnel at ~40MB/s dominates wall
time); PSUM accumulation and the small gate scalars stay fp32.
"""
import sys, os
sys.path.insert(0, '/opt/trn_rl_repo')
import numpy as np

import concourse.bacc as bacc
import concourse.tile as tile
from concourse import mybir
from concourse import bass_utils
from concourse.bass import ds

F32 = mybir.dt.float32
F16 = mybir.dt.float16

I_SZ = 1024
H_SZ = 512
M_SZ = 100
N_IMG = 32768
NC = 8          # cores
B_SCANS = 8     # scans (chunks) per core

# column layout of the fused projection (1280 wide)
C_Z0, C_Z1 = 0, 512        # Whh / Wxh -> Z bank
C_C0, C_C1 = 512, 1024     # Wc -> YC bank
C_S0, C_S1 = 1024, 1280    # small bank: rp[0:100] wp[100:200] rg[200] wg[201] pad
COLS = 1280
S_RP, S_WP, S_RG, S_WG = 0, 100, 200, 201


def build(S_out=512, B_burn=512, T_blk=16, unroll=False):
    """Build the per-core SPMD bass program. Returns nc."""
    n_steps = S_out + B_burn
    assert B_burn <= S_out and B_burn % T_blk == 0 and S_out % T_blk == 0
    xrows_used = B_SCANS * S_out + B_burn
    xrows = ((xrows_used + 127) // 128) * 128

    nc = bacc.Bacc("TRN2", target_bir_lowering=False, debug=False, num_devices=NC)

    xi = nc.dram_tensor("xi", [xrows, I_SZ], F16, kind="ExternalInput")
    xw = nc.dram_tensor("xw", [128, 8, COLS], F16, kind="ExternalInput")
    hw = nc.dram_tensor("hw", [128, 4, COLS], F16, kind="ExternalInput")
    rw = nc.dram_tensor("rw", [128, 4, H_SZ], F16, kind="ExternalInput")
    bias_d = nc.dram_tensor("bias", [1, COLS], F16, kind="ExternalInput")
    ident_d = nc.dram_tensor("ident", [128, 128], F16, kind="ExternalInput")
    colm_d = nc.dram_tensor("colm", [128, B_SCANS, B_SCANS], F16, kind="ExternalInput")
    ones_d = nc.dram_tensor("ones", [1, 128], F16, kind="ExternalInput")
    colmb_d = nc.dram_tensor("colmb", [B_SCANS, B_SCANS, 128], F16, kind="ExternalInput")
    px = nc.dram_tensor("px", [xrows, COLS], F16, kind="Internal")
    out_d = nc.dram_tensor("out", [B_SCANS * S_out, H_SZ], F16, kind="ExternalOutput")

    with tile.TileContext(nc) as tc:
        import contextlib
        with contextlib.ExitStack() as ctx:
            consts = ctx.enter_context(tc.tile_pool(name="consts", bufs=1))
            WH = consts.tile([128, 4, COLS], F16)
            WRH = consts.tile([128, 4, H_SZ], F16)
            BIAS = consts.tile([1, COLS], F16)
            IDENT = consts.tile([128, 128], F16)
            COLM = consts.tile([128, B_SCANS, B_SCANS], F16)
            COLMB = consts.tile([B_SCANS, B_SCANS, 128], F16)
            ONES = consts.tile([1, 128], F16)
            nc.sync.dma_start(out=ONES, in_=ones_d.ap())
            nc.sync.dma_start(out=WH, in_=hw.ap())
            nc.sync.dma_start(out=WRH, in_=rw.ap())
            nc.sync.dma_start(out=BIAS, in_=bias_d.ap())
            nc.sync.dma_start(out=IDENT, in_=ident_d.ap())
            nc.sync.dma_start(out=COLM, in_=colm_d.ap())
            nc.sync.dma_start(out=COLMB, in_=colmb_d.ap())

            # ---------------- phase 1: PX = X @ Wx_all + bias ----------------
            px_stores = []
            n_tchunks = xrows // 128
            with tc.tile_pool(name="p1", bufs=2) as p1, \
                 tc.tile_pool(name="p1w", bufs=1) as p1w, \
                 tc.tile_pool(name="p1ps", bufs=2, space="PSUM") as p1ps, \
                 tc.tile_pool(name="p1pst", bufs=2, space="PSUM") as p1pst:
                XW = p1w.tile([128, 8, COLS], F16)
                nc.sync.dma_start(out=XW, in_=xw.ap())
                for tck in range(n_tchunks):
                    XBLK = p1.tile([128, I_SZ], F16, tag="xblk")
                    nc.sync.dma_start(out=XBLK, in_=xi.ap()[tck * 128:(tck + 1) * 128, :])
                    XT = p1.tile([128, 8, 128], F16, tag="xt")
                    for k in range(8):
                        tp = p1pst.tile([128, 128], F16, tag="tp")
                        nc.tensor.transpose(tp, XBLK[:, k * 128:(k + 1) * 128], IDENT)
                        if k % 2 == 0:
                            nc.vector.tensor_copy(XT[:, k, :], tp)
                        else:
                            nc.scalar.copy(XT[:, k, :], tp)
                    PXB = p1.tile([128, COLS], F16, tag="pxb")
                    for (c0, c1) in ((C_Z0, C_Z1), (C_C0, C_C1), (C_S0, C_S1)):
                        ps = p1ps.tile([128, c1 - c0], F32, tag=f"ps{c0}")
                        for k in range(8):
                            nc.tensor.matmul(ps, XT[:, k, :], XW[:, k, c0:c1],
                                             start=(k == 0), stop=False)
                        nc.tensor.matmul(ps, ONES[0:1, 0:128], BIAS[0:1, c0:c1],
                                         start=False, stop=True)
                        if c0 == C_Z0:
                            nc.vector.tensor_copy(PXB[:, c0:c1], ps)
                        else:
                            nc.scalar.copy(PXB[:, c0:c1], ps)
                    st = nc.sync.dma_start(out=px.ap()[tck * 128:(tck + 1) * 128, :], in_=PXB)
                    px_stores.append(st)

            # ---------------- phase 2: batched scans ----------------
            st_pool = ctx.enter_context(tc.tile_pool(name="state", bufs=1))
            MEMC = st_pool.tile([128, B_SCANS, H_SZ], F16)    # [0:100]=mem, [100]=c row
            ADIAG = st_pool.tile([128, B_SCANS, M_SZ], F16)   # [0:100]=diag
            HT_a = st_pool.tile([128, 4, B_SCANS], F16)
            HT_b = st_pool.tile([128, 4, B_SCANS], F16)
            PXS = st_pool.tile([B_SCANS, T_blk, COLS], F16)
            OUTS_s = st_pool.tile([B_SCANS, T_blk, H_SZ], F16)
            nc.vector.memset(MEMC[0:101, :, :], 0.0)
            nc.vector.memset(HT_a[:, :, :], 0.0)

            ps_pool = ctx.enter_context(tc.tile_pool(name="ps2", bufs=1, space="PSUM"))
            Z_2 = [ps_pool.tile([B_SCANS, H_SZ], F32, tag=f"z{i}", name=f"zps{i}") for i in range(2)]
            YC_ps = ps_pool.tile([B_SCANS, H_SZ], F32, tag="yc")
            YS_ps = ps_pool.tile([B_SCANS, C_S1 - C_S0], F32, tag="ys")
            UPD_ps = [ps_pool.tile([M_SZ, H_SZ], F32, tag=f"upd{i}", name=f"updps{i}") for i in range(2)]
            MISC_ps = [ps_pool.tile([128, 1024], F16, tag=f"misc{i}", name=f"miscps{i}") for i in range(2)]

            sm_pool = ctx.enter_context(tc.tile_pool(name="small", bufs=2))

            def emit_step(s, HT_in, HT_out, OUTS):
                """One scan step for all B_SCANS scans. s = slot in [0, T_blk)."""
                Z_ps = Z_2[s % 2]
                # --- YS matmuls first: they gate the whole step chain ---
                for (c0, c1, ps) in ((C_S0, C_S1, YS_ps),):
                    nc.tensor.matmul(ps, IDENT[0:B_SCANS, 0:B_SCANS],
                                     PXS[:, s, c0:c1], start=True, stop=False)
                    for k in range(4):
                        nc.tensor.matmul(ps, HT_in[:, k, :], WH[:, k, c0:c1],
                                         start=False, stop=(k == 3))
                # --- softmax(ar) first: it gates the critical read chain ---
                AR = sm_pool.tile([B_SCANS, M_SZ], F16, tag="ar")
                SMr = sm_pool.tile([B_SCANS, 1], F32, tag="smr")
                GOS = sm_pool.tile([B_SCANS, 1], F32, tag="gos")
                nc.scalar.activation(AR, YS_ps[:, S_RP:S_RP + M_SZ],
                                     mybir.ActivationFunctionType.Exp,
                                     scale=1.0, accum_out=SMr)
                nc.vector.reciprocal(SMr, SMr)
                # --- gates: go/gw via tanh (one ACT table set with Exp/Relu) ---
                TG = sm_pool.tile([B_SCANS, 2], F32, tag="tg")
                G = sm_pool.tile([B_SCANS, 2], F32, tag="g")
                nc.scalar.activation(TG, YS_ps[:, S_RG:S_WG + 1],
                                     mybir.ActivationFunctionType.Tanh, scale=0.5)
                nc.vector.tensor_scalar(G, TG, 0.5, 0.5,
                                        mybir.AluOpType.mult, mybir.AluOpType.add)
                nc.vector.tensor_scalar(GOS, G[:, 0:1], SMr[:, 0:1], None,
                                        mybir.AluOpType.mult)
                AW = sm_pool.tile([B_SCANS, M_SZ], F16, tag="aw")
                SMw = sm_pool.tile([B_SCANS, 1], F32, tag="smw")
                AWGW = sm_pool.tile([B_SCANS, M_SZ], F16, tag="awgw")
                nc.scalar.activation(AW, YS_ps[:, S_WP:S_WP + M_SZ],
                                     mybir.ActivationFunctionType.Exp,
                                     scale=1.0, accum_out=SMw)
                nc.vector.reciprocal(SMw, SMw)
                nc.vector.tensor_scalar(AW, AW, SMw[:, 0:1], None, mybir.AluOpType.mult)
                nc.vector.tensor_scalar(AWGW, AW, G[:, 1:2], None, mybir.AluOpType.mult)
                MAWGW = sm_pool.tile([B_SCANS, B_SCANS, M_SZ], F16, tag="mawgw")
                nc.vector.tensor_tensor(
                    MAWGW, AWGW.unsqueeze(1).broadcast_to((B_SCANS, B_SCANS, M_SZ)),
                    COLMB[:, :, 0:M_SZ], mybir.AluOpType.mult)
                # --- transpose ar immediately (critical); aw separately later ---
                ART = sm_pool.tile([M_SZ, B_SCANS], F16, tag="art")
                AWT = sm_pool.tile([M_SZ, B_SCANS], F16, tag="awt")
                tpa = MISC_ps[0]
                nc.tensor.transpose(tpa[0:M_SZ, 0:B_SCANS], AR, IDENT[0:B_SCANS, 0:B_SCANS])
                nc.vector.tensor_copy(ART, tpa[0:M_SZ, 0:B_SCANS])
                nc.tensor.transpose(tpa[0:M_SZ, B_SCANS:2 * B_SCANS], AW,
                                    IDENT[0:B_SCANS, 0:B_SCANS])
                nc.vector.tensor_copy(AWT, tpa[0:M_SZ, B_SCANS:2 * B_SCANS])
                # --- masked ar lhsT (one op, critical) ---
                MART = sm_pool.tile([M_SZ, B_SCANS, B_SCANS], F16, tag="mart")
                nc.vector.tensor_tensor(
                    MART, ART.unsqueeze(1).broadcast_to((M_SZ, B_SCANS, B_SCANS)),
                    COLM[0:M_SZ, :, :], mybir.AluOpType.mult)
                W1AWT = sm_pool.tile([M_SZ, B_SCANS], F16, tag="w1awt")
                nc.vector.tensor_scalar(W1AWT, AWT, -1.0, 1.0,
                                        mybir.AluOpType.mult, mybir.AluOpType.add)
                nc.vector.tensor_tensor(
                    ADIAG[0:M_SZ, :, :],
                    IDENT[0:M_SZ, 0:M_SZ].unsqueeze(1).broadcast_to((M_SZ, B_SCANS, M_SZ)),
                    W1AWT.unsqueeze(2).broadcast_to((M_SZ, B_SCANS, M_SZ)),
                    mybir.AluOpType.mult)
                # --- gated memory read: RRAW[j] = ar_j @ mem_j ---
                RR = MISC_ps[1].bitcast(F32)
                for j in range(B_SCANS):
                    nc.tensor.matmul(RR[0:B_SCANS, 0:H_SZ], MART[:, j, :],
                                     MEMC[0:M_SZ, j, :],
                                     start=(j == 0), stop=(j == B_SCANS - 1))
                R = sm_pool.tile([B_SCANS, H_SZ], F16, tag="r")
                nc.vector.tensor_scalar(R, RR[0:B_SCANS, 0:H_SZ], GOS[:, 0:1], None,
                                        mybir.AluOpType.mult)
                # --- YC and Z streams (filler priority; Z group stays open for Wrh) ---
                for (c0, c1, ps) in ((C_C0, C_C1, YC_ps), (C_Z0, C_Z1, Z_ps)):
                    nc.tensor.matmul(ps, IDENT[0:B_SCANS, 0:B_SCANS],
                                     PXS[:, s, c0:c1], start=True, stop=False)
                    last = (c0 != C_Z0)
                    for k in range(4):
                        nc.tensor.matmul(ps, HT_in[:, k, :], WH[:, k, c0:c1],
                                         start=False, stop=(last and k == 3))
                C = sm_pool.tile([B_SCANS, H_SZ], F16, tag="c")
                nc.scalar.activation(C, YC_ps, mybir.ActivationFunctionType.Relu)
                # --- R^T (4 transposes into one bank, one copy); Z += R @ Wrh ---
                RT = sm_pool.tile([128, 4, B_SCANS], F16, tag="rt")
                tpr = MISC_ps[1]
                for k in range(4):
                    nc.tensor.transpose(tpr[:, k * B_SCANS:(k + 1) * B_SCANS],
                                        R[:, k * 128:(k + 1) * 128],
                                        IDENT[0:B_SCANS, 0:B_SCANS])
                nc.vector.tensor_copy(RT, tpr[:, 0:4 * B_SCANS])
                for k in range(4):
                    nc.tensor.matmul(Z_ps, RT[:, k, :], WRH[:, k, :],
                                     start=False, stop=(k == 3))
                # --- h_new ---
                nc.scalar.activation(OUTS[:, s, :], Z_ps, mybir.ActivationFunctionType.Relu)
                # --- memory update: mem = diag(1-aw) mem + awgw (x) c ---
                for j in range(B_SCANS):
                    ups = UPD_ps[j % 2]
                    nc.tensor.matmul(ups, ADIAG[0:M_SZ, j, :],
                                     MEMC[0:M_SZ, j, :], start=True, stop=False)
                    nc.tensor.matmul(ups, MAWGW[:, j, :], C,
                                     start=False, stop=True)
                    if j % 2 == 0:
                        nc.scalar.copy(MEMC[0:M_SZ, j, :], ups)
                    else:
                        nc.vector.tensor_copy(MEMC[0:M_SZ, j, :], ups)

                # --- H^T for next step (4 transposes, one copy) ---
                tph = MISC_ps[0]
                for k in range(4):
                    nc.tensor.transpose(tph[:, k * B_SCANS:(k + 1) * B_SCANS],
                                        OUTS[:, s, k * 128:(k + 1) * 128],
                                        IDENT[0:B_SCANS, 0:B_SCANS])
                nc.vector.tensor_copy(HT_out[:, :, :], tph[:, 0:4 * B_SCANS])

            pxA = px.ap()[0:B_SCANS * S_out, :].rearrange("(a t) n -> a t n", t=S_out)
            pxB = px.ap()[B_burn:B_burn + B_SCANS * S_out, :].rearrange("(a t) n -> a t n", t=S_out)
            outv = out_d.ap().rearrange("(j t) h -> j t h", t=S_out)

            def body_burn(i):
                ldA = nc.sync.dma_start(out=PXS, in_=pxA[0:B_SCANS, :, :][:, ds(i, T_blk), :])
                for st in px_stores:
                    tile.add_dep_helper(ldA.ins, st.ins, reason="phase1 px ready")
                for s in range(T_blk):
                    HT_in = HT_a if s % 2 == 0 else HT_b
                    HT_out = HT_b if s % 2 == 0 else HT_a
                    emit_step(s, HT_in, HT_out, OUTS_s)

            def body_out(i):
                ldB = nc.sync.dma_start(out=PXS, in_=pxB[:, ds(i, T_blk), :])
                for st in px_stores:
                    tile.add_dep_helper(ldB.ins, st.ins, reason="phase1 px ready")
                for s in range(T_blk):
                    HT_in = HT_a if s % 2 == 0 else HT_b
                    HT_out = HT_b if s % 2 == 0 else HT_a
                    emit_step(s, HT_in, HT_out, OUTS_s)
                nc.sync.dma_start(out=outv[:, ds(i, T_blk), :], in_=OUTS_s)

            if unroll:
                for i in range(0, B_burn, T_blk):
                    body_burn(i)
                for i in range(0, S_out, T_blk):
                    body_out(i)
            else:
                hints = (mybir.EngineType.PE, mybir.EngineType.DVE,
                         mybir.EngineType.Activation, mybir.EngineType.SP)
                with tc.For_i(0, B_burn, T_blk, hint_engines=hints) as i:
                    body_burn(i)
                with tc.For_i(0, S_out, T_blk, hint_engines=hints) as i:
                    body_out(i)

    nc.compile()
    return nc


def make_inputs_per_core(hidden_frames, Wc, bc, Wwg, bwg, Wwp, bwp, Wrg, brg,
                         Wrp, brp, Wxh, Wrh, Whh, bh, S_out=512, B_burn=512):
    I, H, M = I_SZ, H_SZ, M_SZ
    Wx_all = np.zeros((I, COLS), np.float32)
    Wh_all = np.zeros((H, COLS), np.float32)
    bias_all = np.zeros((1, COLS), np.float32)
    Wx_all[:, C_Z0:C_Z1] = Wxh
    Wh_all[:, C_Z0:C_Z1] = Whh
    Wx_all[:, C_C0:C_C1] = Wc[:I]
    Wh_all[:, C_C0:C_C1] = Wc[I:]
    Wx_all[:, C_S0 + S_RP:C_S0 + S_RP + M] = Wrp[:I]
    Wh_all[:, C_S0 + S_RP:C_S0 + S_RP + M] = Wrp[I:]
    Wx_all[:, C_S0 + S_WP:C_S0 + S_WP + M] = Wwp[:I]
    Wh_all[:, C_S0 + S_WP:C_S0 + S_WP + M] = Wwp[I:]
    Wx_all[:, C_S0 + S_RG] = Wrg[:I, 0]
    Wh_all[:, C_S0 + S_RG] = Wrg[I:, 0]
    Wx_all[:, C_S0 + S_WG] = Wwg[:I, 0]
    Wh_all[:, C_S0 + S_WG] = Wwg[I:, 0]
    bias_all[0, C_Z0:C_Z1] = bh
    bias_all[0, C_C0:C_C1] = bc
    bias_all[0, C_S0 + S_RP:C_S0 + S_RP + M] = brp
    bias_all[0, C_S0 + S_WP:C_S0 + S_WP + M] = bwp
    bias_all[0, C_S0 + S_RG] = np.float32(np.asarray(brg).reshape(-1)[0])
    bias_all[0, C_S0 + S_WG] = np.float32(np.asarray(bwg).reshape(-1)[0])

    f16 = np.float16
    xw = np.ascontiguousarray(
        Wx_all.reshape(8, 128, COLS).transpose(1, 0, 2)).astype(f16)
    hww = np.ascontiguousarray(
        Wh_all.reshape(4, 128, COLS).transpose(1, 0, 2)).astype(f16)
    rww = np.ascontiguousarray(
        Wrh.astype(np.float32).reshape(4, 128, H).transpose(1, 0, 2)).astype(f16)
    ident = np.eye(128, dtype=f16)
    colm = np.zeros((128, B_SCANS, B_SCANS), f16)
    for j in range(B_SCANS):
        colm[:, j, j] = 1.0
    colmb = np.zeros((B_SCANS, B_SCANS, 128), f16)
    for j in range(B_SCANS):
        colmb[j, j, :] = 1.0

    xrows_used = B_SCANS * S_out + B_burn
    xrows = ((xrows_used + 127) // 128) * 128
    T = hidden_frames.shape[0]
    per_core = B_SCANS * S_out
    X16 = hidden_frames.astype(f16)
    in_maps = []
    for c in range(NC):
        lo = c * per_core - B_burn  # may be negative for core 0
        xi = np.zeros((xrows, I), f16)
        src0 = max(lo, 0)
        src1 = min(lo + xrows, T)
        if src1 > src0:
            xi[src0 - lo:src1 - lo] = X16[src0:src1]
        in_maps.append({"xi": xi, "xw": xw, "hw": hww, "rw": rww,
                        "bias": bias_all.astype(f16), "ident": ident,
                        "colm": colm,
                        "ones": np.ones((1, 128), f16),
                        "colmb": colmb})
    return in_maps


_BUILT = {}


def kernel(hidden_frames, Wc, bc, Wwg, bwg, Wwp, bwp, Wrg, brg, Wrp, brp,
           Wxh, Wrh, Whh, bh, nImg):
    assert int(nImg) == N_IMG
    S_out, B_burn = 512, 384
    key = (S_out, B_burn)
    if key not in _BUILT:
        _BUILT[key] = build(S_out=S_out, B_burn=B_burn)
    nc = _BUILT[key]
    in_maps = make_inputs_per_core(
        np.asarray(hidden_frames), np.asarray(Wc), np.asarray(bc),
        np.asarray(Wwg), np.asarray(bwg), np.asarray(Wwp), np.asarray(bwp),
        np.asarray(Wrg), np.asarray(brg), np.asarray(Wrp), np.asarray(brp),
        np.asarray(Wxh), np.asarray(Wrh), np.asarray(Whh), np.asarray(bh),
        S_out=S_out, B_burn=B_burn)
    res = bass_utils.run_bass_kernel_spmd(nc, in_maps, core_ids=list(range(NC)))
    return np.concatenate([res.results[c]["out"] for c in range(NC)],
                          axis=0).astype(np.float32)


# revision 5
# speedup vs baseline: 1.4573x; 1.4573x over previous
"""Trainium2 Bass kernel for nn_MemoryRamModule (scatter_memory).

Strategy: the reference is a strictly-sequential 32768-step scan with a
(mem[100,512], h[512]) carry, but the memory decays per step by (1-aw),
aw ~ softmax ~ 1/100, so carry influence dies off as e^(-0.01*B). We split
time into 64 chunks of 512 steps, run 8 independent chunk-scans per core
(batched), each with a burn-in re-deriving the carry. Scan g reads input
rows [g*512-B_burn, g*512+512), zero-padded below row 0 (zero inputs
provably keep the carry exactly zero), and emits its last 512 steps as
output rows [g*512, (g+1)*512).

Per core: phase 1 projects its X slab through all x-side weight columns
(one big matmul -> PX in DRAM); phase 2 runs the 8 scans batched, with the
per-step recurrent work done as small PE matmuls (h-projections, gated
memory read, rank-1 + decay memory update) plus DVE/ACT softmax/gate ops.

Host<->device IO is minimized (the axon tunnel at ~60MB/s dominates wall
time): X ships as int8 with per-row scales, the output ships as uint8
with per-row scales, and the tiny replicated weights ship sharded 1/8th
per core and are AllGathered on-device. Compute is fp16 with fp32 PSUM.
"""
import sys, os
sys.path.insert(0, '/opt/trn_rl_repo')
import numpy as np

import concourse.bacc as bacc
import concourse.tile as tile
from concourse import mybir
from concourse import bass_utils
from concourse.bass import ds

F32 = mybir.dt.float32
F16 = mybir.dt.float16
I8 = mybir.dt.int8
U8 = mybir.dt.uint8

I_SZ = 1024
H_SZ = 512
M_SZ = 100
N_IMG = 32768
NC = 8          # cores
B_SCANS = 8     # scans (chunks) per core

# column layout of the fused projection (1280 wide)
C_Z0, C_Z1 = 0, 512        # Whh / Wxh -> Z bank
C_C0, C_C1 = 512, 1024     # Wc -> YC bank
C_S0, C_S1 = 1024, 1280    # small bank: rp[0:100] wp[100:200] rg[200] wg[201] pad
COLS = 1280
S_RP, S_WP, S_RG, S_WG = 0, 100, 200, 201

# packed-weights blob layout, f16 rows of 1024 (AllGathered on device)
OFF_XW, N_XW = 0, 1280          # [128,8,1280]
OFF_HW, N_HW = 1280, 640        # [128,4,1280]
OFF_RW, N_RW = 1920, 256        # [128,4,512]
OFF_BIAS, N_BIAS = 2176, 2      # [1,1280] (+pad)
OFF_ID, N_ID = 2178, 16         # [128,128]
OFF_CM, N_CM = 2194, 8          # [128,8,8]
OFF_CB, N_CB = 2202, 8          # [8,8,128]
WROWS = 2216                    # padded to NC*277
WSHARD = WROWS // NC

QOUT = 254.0                    # uint8 quant full-scale


def build(S_out=512, B_burn=512, T_blk=16, unroll=False):
    """Build the per-core SPMD bass program. Returns nc."""
    assert B_burn <= S_out and B_burn % T_blk == 0 and S_out % T_blk == 0
    xrows_used = B_SCANS * S_out + B_burn
    xrows = ((xrows_used + 127) // 128) * 128

    nc = bacc.Bacc("TRN2", target_bir_lowering=False, debug=False, num_devices=NC)

    xq = nc.dram_tensor("xq", [xrows, I_SZ], I8, kind="ExternalInput")
    xs = nc.dram_tensor("xs", [xrows, 1], F32, kind="ExternalInput")
    wpack = nc.dram_tensor("wpack", [WSHARD, 1024], F16, kind="ExternalInput")
    wstage = nc.dram_tensor("wstage", [WSHARD, 1024], F16, kind="Internal")
    wfull = nc.dram_tensor("wfull", [WROWS, 1024], F16, kind="Internal")
    px = nc.dram_tensor("px", [xrows, COLS], F16, kind="Internal")
    outq_d = nc.dram_tensor("outq", [B_SCANS * S_out, H_SZ], U8, kind="ExternalOutput")
    outsc_d = nc.dram_tensor("outsc", [B_SCANS * S_out, 1], F16, kind="ExternalOutput")

    with tile.TileContext(nc) as tc:
        import contextlib
        with contextlib.ExitStack() as ctx:
            # on-device weight AllGather: each core contributes 1/NC of blob
            # (collectives can't read IO tensors, so stage through Internal)
            ld0 = nc.sync.dma_start(out=wstage.ap(), in_=wpack.ap())
            cc = nc.gpsimd.collective_compute(
                kind="AllGather", op=mybir.AluOpType.bypass,
                replica_groups=[list(range(NC))],
                ins=[wstage.ap()], outs=[wfull.ap()])
            tile.add_dep_helper(cc.ins, ld0.ins, reason="stage wpack")
            wf = wfull.ap()

            consts = ctx.enter_context(tc.tile_pool(name="consts", bufs=1))
            WH = consts.tile([128, 4, COLS], F16)
            WRH = consts.tile([128, 4, H_SZ], F16)
            BIAS = consts.tile([1, COLS], F16)
            IDENT = consts.tile([128, 128], F16)
            COLM = consts.tile([128, B_SCANS, B_SCANS], F16)
            COLMB = consts.tile([B_SCANS, B_SCANS, 128], F16)
            ONES = consts.tile([1, 128], F16)
            nc.vector.memset(ONES, 1.0)
            wloads = [
                nc.sync.dma_start(out=WH, in_=wf[OFF_HW:OFF_HW + N_HW, :]
                                  .rearrange("(p r) c -> p (r c)", r=5)
                                  .rearrange("p (a b) -> p a b", a=4)),
                nc.sync.dma_start(out=WRH, in_=wf[OFF_RW:OFF_RW + N_RW, :]
                                  .rearrange("(p r) c -> p (r c)", r=2)
                                  .rearrange("p (a b) -> p a b", a=4)),
                nc.sync.dma_start(out=BIAS[0:1, 0:1024],
                                  in_=wf[OFF_BIAS:OFF_BIAS + 1, :]),
                nc.sync.dma_start(out=BIAS[0:1, 1024:COLS],
                                  in_=wf[OFF_BIAS + 1:OFF_BIAS + 2, 0:COLS - 1024]),
                nc.sync.dma_start(out=IDENT, in_=wf[OFF_ID:OFF_ID + N_ID, :]
                                  .rearrange("r (e c) -> (r e) c", c=128)),
                nc.sync.dma_start(out=COLM, in_=wf[OFF_CM:OFF_CM + N_CM, :]
                                  .rearrange("r (e c) -> (r e) c", c=64)
                                  .rearrange("p (a b) -> p a b", a=B_SCANS)),
                nc.sync.dma_start(out=COLMB, in_=wf[OFF_CB:OFF_CB + N_CB, :]
                                  .rearrange("r (a b) -> r a b", a=B_SCANS)),
            ]
            for ld in wloads:
                tile.add_dep_helper(ld.ins, cc.ins, reason="allgather weights")

            # ---------------- phase 1: PX = X @ Wx_all + bias ----------------
            px_stores = []
            n_tchunks = xrows // 128
            with tc.tile_pool(name="p1", bufs=2) as p1, \
                 tc.tile_pool(name="p1w", bufs=1) as p1w, \
                 tc.tile_pool(name="p1ps", bufs=2, space="PSUM") as p1ps, \
                 tc.tile_pool(name="p1pst", bufs=2, space="PSUM") as p1pst:
                XW = p1w.tile([128, 8, COLS], F16)
                ldxw = nc.sync.dma_start(out=XW, in_=wf[OFF_XW:OFF_XW + N_XW, :]
                                         .rearrange("(p r) c -> p (r c)", r=10)
                                         .rearrange("p (a b) -> p a b", a=8))
                tile.add_dep_helper(ldxw.ins, cc.ins, reason="allgather weights")
                for tck in range(n_tchunks):
                    XQB = p1.tile([128, I_SZ], I8, tag="xqb")
                    XSC = p1.tile([128, 1], F32, tag="xsc")
                    nc.sync.dma_start(out=XQB, in_=xq.ap()[tck * 128:(tck + 1) * 128, :])
                    nc.sync.dma_start(out=XSC, in_=xs.ap()[tck * 128:(tck + 1) * 128, :])
                    XBLK = p1.tile([128, I_SZ], F16, tag="xblk")
                    nc.scalar.activation(XBLK, XQB,
                                         mybir.ActivationFunctionType.Copy,
                                         scale=XSC[:, 0:1])
                    XT = p1.tile([128, 8, 128], F16, tag="xt")
                    for k in range(8):
                        tp = p1pst.tile([128, 128], F16, tag="tp")
                        nc.tensor.transpose(tp, XBLK[:, k * 128:(k + 1) * 128], IDENT)
                        if k % 2 == 0:
                            nc.vector.tensor_copy(XT[:, k, :], tp)
                        else:
                            nc.scalar.copy(XT[:, k, :], tp)
                    PXB = p1.tile([128, COLS], F16, tag="pxb")
                    for (c0, c1) in ((C_Z0, C_Z1), (C_C0, C_C1), (C_S0, C_S1)):
                        ps = p1ps.tile([128, c1 - c0], F32, tag=f"ps{c0}")
                        for k in range(8):
                            nc.tensor.matmul(ps, XT[:, k, :], XW[:, k, c0:c1],
                                             start=(k == 0), stop=False)
                        nc.tensor.matmul(ps, ONES[0:1, 0:128], BIAS[0:1, c0:c1],
                                         start=False, stop=True)
                        if c0 == C_Z0:
                            nc.vector.tensor_copy(PXB[:, c0:c1], ps)
                        else:
                            nc.scalar.copy(PXB[:, c0:c1], ps)
                    st = nc.sync.dma_start(out=px.ap()[tck * 128:(tck + 1) * 128, :], in_=PXB)
                    px_stores.append(st)

            # ---------------- phase 2: batched scans ----------------
            st_pool = ctx.enter_context(tc.tile_pool(name="state", bufs=1))
            MEMC = st_pool.tile([128, B_SCANS, H_SZ], F16)    # [0:100]=mem
            ADIAG = st_pool.tile([128, B_SCANS, M_SZ], F16)   # [0:100]=diag
            HT_a = st_pool.tile([128, 4, B_SCANS], F16)
            HT_b = st_pool.tile([128, 4, B_SCANS], F16)
            PXS = st_pool.tile([B_SCANS, T_blk, COLS], F16)
            OUTS_s = st_pool.tile([B_SCANS, T_blk, H_SZ], F16)
            OUTQ_s = st_pool.tile([B_SCANS, T_blk, H_SZ], U8)
            OUTSC_s = st_pool.tile([B_SCANS, T_blk, 1], F16)
            nc.vector.memset(MEMC[0:101, :, :], 0.0)
            nc.vector.memset(HT_a[:, :, :], 0.0)

            ps_pool = ctx.enter_context(tc.tile_pool(name="ps2", bufs=1, space="PSUM"))
            Z_2 = [ps_pool.tile([B_SCANS, H_SZ], F32, tag=f"z{i}", name=f"zps{i}") for i in range(2)]
            YC_ps = ps_pool.tile([B_SCANS, H_SZ], F32, tag="yc")
            YS_ps = ps_pool.tile([B_SCANS, C_S1 - C_S0], F32, tag="ys")
            UPD_ps = [ps_pool.tile([M_SZ, H_SZ], F32, tag=f"upd{i}", name=f"updps{i}") for i in range(2)]
            MISC_ps = [ps_pool.tile([128, 1024], F16, tag=f"misc{i}", name=f"miscps{i}") for i in range(2)]

            sm_pool = ctx.enter_context(tc.tile_pool(name="small", bufs=2))

            def emit_step(s, HT_in, HT_out, OUTS, quant):
                """One scan step for all B_SCANS scans. s = slot in [0, T_blk)."""
                Z_ps = Z_2[s % 2]
                # --- YS matmuls first: they gate the whole step chain ---
                for (c0, c1, ps) in ((C_S0, C_S1, YS_ps),):
                    nc.tensor.matmul(ps, IDENT[0:B_SCANS, 0:B_SCANS],
                                     PXS[:, s, c0:c1], start=True, stop=False)
                    for k in range(4):
                        nc.tensor.matmul(ps, HT_in[:, k, :], WH[:, k, c0:c1],
                                         start=False, stop=(k == 3))
                # --- softmax(ar) first: it gates the critical read chain ---
                AR = sm_pool.tile([B_SCANS, M_SZ], F16, tag="ar")
                SMr = sm_pool.tile([B_SCANS, 1], F32, tag="smr")
                GOS = sm_pool.tile([B_SCANS, 1], F32, tag="gos")
                nc.scalar.activation(AR, YS_ps[:, S_RP:S_RP + M_SZ],
                                     mybir.ActivationFunctionType.Exp,
                                     scale=1.0, accum_out=SMr)
                nc.vector.reciprocal(SMr, SMr)
                # --- gates: go/gw via tanh (one ACT table set with Exp/Relu) ---
                TG = sm_pool.tile([B_SCANS, 2], F32, tag="tg")
                G = sm_pool.tile([B_SCANS, 2], F32, tag="g")
                nc.scalar.activation(TG, YS_ps[:, S_RG:S_WG + 1],
                                     mybir.ActivationFunctionType.Tanh, scale=0.5)
                nc.vector.tensor_scalar(G, TG, 0.5, 0.5,
                                        mybir.AluOpType.mult, mybir.AluOpType.add)
                nc.vector.tensor_scalar(GOS, G[:, 0:1], SMr[:, 0:1], None,
                                        mybir.AluOpType.mult)
                AW = sm_pool.tile([B_SCANS, M_SZ], F16, tag="aw")
                SMw = sm_pool.tile([B_SCANS, 1], F32, tag="smw")
                AWGW = sm_pool.tile([B_SCANS, M_SZ], F16, tag="awgw")
                nc.scalar.activation(AW, YS_ps[:, S_WP:S_WP + M_SZ],
                                     mybir.ActivationFunctionType.Exp,
                                     scale=1.0, accum_out=SMw)
                nc.vector.reciprocal(SMw, SMw)
                nc.vector.tensor_scalar(AW, AW, SMw[:, 0:1], None, mybir.AluOpType.mult)
                nc.vector.tensor_scalar(AWGW, AW, G[:, 1:2], None, mybir.AluOpType.mult)
                MAWGW = sm_pool.tile([B_SCANS, B_SCANS, M_SZ], F16, tag="mawgw")
                nc.vector.tensor_tensor(
                    MAWGW, AWGW.unsqueeze(1).broadcast_to((B_SCANS, B_SCANS, M_SZ)),
                    COLMB[:, :, 0:M_SZ], mybir.AluOpType.mult)
                # --- transpose ar immediately (critical); aw separately later ---
                ART = sm_pool.tile([M_SZ, B_SCANS], F16, tag="art")
                AWT = sm_pool.tile([M_SZ, B_SCANS], F16, tag="awt")
                tpa = MISC_ps[0]
                nc.tensor.transpose(tpa[0:M_SZ, 0:B_SCANS], AR, IDENT[0:B_SCANS, 0:B_SCANS])
                nc.vector.tensor_copy(ART, tpa[0:M_SZ, 0:B_SCANS])
                nc.tensor.transpose(tpa[0:M_SZ, B_SCANS:2 * B_SCANS], AW,
                                    IDENT[0:B_SCANS, 0:B_SCANS])
                nc.vector.tensor_copy(AWT, tpa[0:M_SZ, B_SCANS:2 * B_SCANS])
                # --- masked ar lhsT (one op, critical) ---
                MART = sm_pool.tile([M_SZ, B_SCANS, B_SCANS], F16, tag="mart")
                nc.vector.tensor_tensor(
                    MART, ART.unsqueeze(1).broadcast_to((M_SZ, B_SCANS, B_SCANS)),
                    COLM[0:M_SZ, :, :], mybir.AluOpType.mult)
                W1AWT = sm_pool.tile([M_SZ, B_SCANS], F16, tag="w1awt")
                nc.vector.tensor_scalar(W1AWT, AWT, -1.0, 1.0,
                                        mybir.AluOpType.mult, mybir.AluOpType.add)
                nc.vector.tensor_tensor(
                    ADIAG[0:M_SZ, :, :],
                    IDENT[0:M_SZ, 0:M_SZ].unsqueeze(1).broadcast_to((M_SZ, B_SCANS, M_SZ)),
                    W1AWT.unsqueeze(2).broadcast_to((M_SZ, B_SCANS, M_SZ)),
                    mybir.AluOpType.mult)
                # --- gated memory read: RRAW[j] = ar_j @ mem_j ---
                RR = MISC_ps[1].bitcast(F32)
                for j in range(B_SCANS):
                    nc.tensor.matmul(RR[0:B_SCANS, 0:H_SZ], MART[:, j, :],
                                     MEMC[0:M_SZ, j, :],
                                     start=(j == 0), stop=(j == B_SCANS - 1))
                R = sm_pool.tile([B_SCANS, H_SZ], F16, tag="r")
                nc.vector.tensor_scalar(R, RR[0:B_SCANS, 0:H_SZ], GOS[:, 0:1], None,
                                        mybir.AluOpType.mult)
                # --- YC and Z streams (filler priority; Z group stays open for Wrh) ---
                for (c0, c1, ps) in ((C_C0, C_C1, YC_ps), (C_Z0, C_Z1, Z_ps)):
                    nc.tensor.matmul(ps, IDENT[0:B_SCANS, 0:B_SCANS],
                                     PXS[:, s, c0:c1], start=True, stop=False)
                    last = (c0 != C_Z0)
                    for k in range(4):
                        nc.tensor.matmul(ps, HT_in[:, k, :], WH[:, k, c0:c1],
                                         start=False, stop=(last and k == 3))
                C = sm_pool.tile([B_SCANS, H_SZ], F16, tag="c")
                nc.scalar.activation(C, YC_ps, mybir.ActivationFunctionType.Relu)
                # --- R^T (4 transposes into one bank, one copy); Z += R @ Wrh ---
                RT = sm_pool.tile([128, 4, B_SCANS], F16, tag="rt")
                tpr = MISC_ps[1]
                for k in range(4):
                    nc.tensor.transpose(tpr[:, k * B_SCANS:(k + 1) * B_SCANS],
                                        R[:, k * 128:(k + 1) * 128],
                                        IDENT[0:B_SCANS, 0:B_SCANS])
                nc.vector.tensor_copy(RT, tpr[:, 0:4 * B_SCANS])
                for k in range(4):
                    nc.tensor.matmul(Z_ps, RT[:, k, :], WRH[:, k, :],
                                     start=False, stop=(k == 3))
                # --- h_new ---
                nc.scalar.activation(OUTS[:, s, :], Z_ps, mybir.ActivationFunctionType.Relu)
                # --- quantize h row to uint8 with per-row scale (output steps) ---
                if quant:
                    RMX = sm_pool.tile([B_SCANS, 1], F32, tag="rmx")
                    RSC = sm_pool.tile([B_SCANS, 1], F32, tag="rsc")
                    nc.vector.reduce_max(RMX, OUTS[:, s, :], axis=mybir.AxisListType.X)
                    nc.vector.tensor_scalar(RMX, RMX, 1.0 / QOUT, 1e-7,
                                            mybir.AluOpType.mult, mybir.AluOpType.max)
                    nc.vector.reciprocal(RSC, RMX)
                    nc.vector.tensor_scalar(OUTQ_s[:, s, :], OUTS[:, s, :],
                                            RSC[:, 0:1], 0.5,
                                            mybir.AluOpType.mult, mybir.AluOpType.add)
                    nc.scalar.copy(OUTSC_s[:, s, 0:1], RMX)
                # --- memory update: mem = diag(1-aw) mem + awgw (x) c ---
                for j in range(B_SCANS):
                    ups = UPD_ps[j % 2]
                    nc.tensor.matmul(ups, ADIAG[0:M_SZ, j, :],
                                     MEMC[0:M_SZ, j, :], start=True, stop=False)
                    nc.tensor.matmul(ups, MAWGW[:, j, :], C,
                                     start=False, stop=True)
                    if j % 2 == 0:
                        nc.scalar.copy(MEMC[0:M_SZ, j, :], ups)
                    else:
                        nc.vector.tensor_copy(MEMC[0:M_SZ, j, :], ups)

                # --- H^T for next step (4 transposes, one copy) ---
                tph = MISC_ps[0]
                for k in range(4):
                    nc.tensor.transpose(tph[:, k * B_SCANS:(k + 1) * B_SCANS],
                                        OUTS[:, s, k * 128:(k + 1) * 128],
                                        IDENT[0:B_SCANS, 0:B_SCANS])
                nc.vector.tensor_copy(HT_out[:, :, :], tph[:, 0:4 * B_SCANS])

            pxA = px.ap()[0:B_SCANS * S_out, :].rearrange("(a t) n -> a t n", t=S_out)
            pxB = px.ap()[B_burn:B_burn + B_SCANS * S_out, :].rearrange("(a t) n -> a t n", t=S_out)
            outqv = outq_d.ap().rearrange("(j t) h -> j t h", t=S_out)
            outscv = outsc_d.ap().rearrange("(j t) h -> j t h", t=S_out)

            def body_burn(i):
                ldA = nc.sync.dma_start(out=PXS, in_=pxA[0:B_SCANS, :, :][:, ds(i, T_blk), :])
                for st in px_stores:
                    tile.add_dep_helper(ldA.ins, st.ins, reason="phase1 px ready")
                for s in range(T_blk):
                    HT_in = HT_a if s % 2 == 0 else HT_b
                    HT_out = HT_b if s % 2 == 0 else HT_a
                    emit_step(s, HT_in, HT_out, OUTS_s, quant=False)

            def body_out(i):
                ldB = nc.sync.dma_start(out=PXS, in_=pxB[:, ds(i, T_blk), :])
                for st in px_stores:
                    tile.add_dep_helper(ldB.ins, st.ins, reason="phase1 px ready")
                for s in range(T_blk):
                    HT_in = HT_a if s % 2 == 0 else HT_b
                    HT_out = HT_b if s % 2 == 0 else HT_a
                    emit_step(s, HT_in, HT_out, OUTS_s, quant=True)
                nc.sync.dma_start(out=outqv[:, ds(i, T_blk), :], in_=OUTQ_s)
                nc.sync.dma_start(out=outscv[:, ds(i, T_blk), :], in_=OUTSC_s)

            if unroll:
                for i in range(0, B_burn, T_blk):
                    body_burn(i)
                for i in range(0, S_out, T_blk):
                    body_out(i)
            else:
                hints = (mybir.EngineType.PE, mybir.EngineType.DVE,
                         mybir.EngineType.Activation, mybir.EngineType.SP)
                with tc.For_i(0, B_burn, T_blk, hint_engines=hints) as i:
                    body_burn(i)
                with tc.For_i(0, S_out, T_blk, hint_engines=hints) as i:
                    body_out(i)

    nc.compile()
    return nc


def pack_weights(Wc, bc, Wwg, bwg, Wwp, bwp, Wrg, brg, Wrp, brp,
                 Wxh, Wrh, Whh, bh):
    I, H, M = I_SZ, H_SZ, M_SZ
    Wx_all = np.zeros((I, COLS), np.float32)
    Wh_all = np.zeros((H, COLS), np.float32)
    bias_all = np.zeros((1, COLS), np.float32)
    Wx_all[:, C_Z0:C_Z1] = Wxh
    Wh_all[:, C_Z0:C_Z1] = Whh
    Wx_all[:, C_C0:C_C1] = Wc[:I]
    Wh_all[:, C_C0:C_C1] = Wc[I:]
    Wx_all[:, C_S0 + S_RP:C_S0 + S_RP + M] = Wrp[:I]
    Wh_all[:, C_S0 + S_RP:C_S0 + S_RP + M] = Wrp[I:]
    Wx_all[:, C_S0 + S_WP:C_S0 + S_WP + M] = Wwp[:I]
    Wh_all[:, C_S0 + S_WP:C_S0 + S_WP + M] = Wwp[I:]
    Wx_all[:, C_S0 + S_RG] = Wrg[:I, 0]
    Wh_all[:, C_S0 + S_RG] = Wrg[I:, 0]
    Wx_all[:, C_S0 + S_WG] = Wwg[:I, 0]
    Wh_all[:, C_S0 + S_WG] = Wwg[I:, 0]
    bias_all[0, C_Z0:C_Z1] = bh
    bias_all[0, C_C0:C_C1] = bc
    bias_all[0, C_S0 + S_RP:C_S0 + S_RP + M] = brp
    bias_all[0, C_S0 + S_WP:C_S0 + S_WP + M] = bwp
    bias_all[0, C_S0 + S_RG] = np.float32(np.asarray(brg).reshape(-1)[0])
    bias_all[0, C_S0 + S_WG] = np.float32(np.asarray(bwg).reshape(-1)[0])

    f16 = np.float16
    xw = np.ascontiguousarray(
        Wx_all.reshape(8, 128, COLS).transpose(1, 0, 2)).astype(f16)
    hww = np.ascontiguousarray(
        Wh_all.reshape(4, 128, COLS).transpose(1, 0, 2)).astype(f16)
    rww = np.ascontiguousarray(
        Wrh.astype(np.float32).reshape(4, 128, H).transpose(1, 0, 2)).astype(f16)
    ident = np.eye(128, dtype=f16)
    colm = np.zeros((128, B_SCANS, B_SCANS), f16)
    for j in range(B_SCANS):
        colm[:, j, j] = 1.0
    colmb = np.zeros((B_SCANS, B_SCANS, 128), f16)
    for j in range(B_SCANS):
        colmb[j, j, :] = 1.0

    blob = np.zeros((WROWS, 1024), f16)
    blob[OFF_XW:OFF_XW + N_XW] = xw.reshape(N_XW, 1024)
    blob[OFF_HW:OFF_HW + N_HW] = hww.reshape(N_HW, 1024)
    blob[OFF_RW:OFF_RW + N_RW] = rww.reshape(N_RW, 1024)
    bias16 = bias_all.astype(f16).reshape(-1)
    blob[OFF_BIAS, :1024] = bias16[:1024]
    blob[OFF_BIAS + 1, :COLS - 1024] = bias16[1024:]
    blob[OFF_ID:OFF_ID + N_ID] = ident.reshape(N_ID, 1024)
    blob[OFF_CM:OFF_CM + N_CM] = colm.reshape(N_CM, 1024)
    blob[OFF_CB:OFF_CB + N_CB] = colmb.reshape(N_CB, 1024)
    return blob


def make_inputs_per_core(hidden_frames, Wc, bc, Wwg, bwg, Wwp, bwp, Wrg, brg,
                         Wrp, brp, Wxh, Wrh, Whh, bh, S_out=512, B_burn=512):
    blob = pack_weights(Wc, bc, Wwg, bwg, Wwp, bwp, Wrg, brg, Wrp, brp,
                        Wxh, Wrh, Whh, bh)

    X = np.asarray(hidden_frames, np.float32)
    T = X.shape[0]
    amax = np.abs(X).max(axis=1)
    xs_full = (np.maximum(amax, 1e-9) / 127.0).astype(np.float32)
    tmp = X * (np.float32(1.0) / xs_full)[:, None]
    np.rint(tmp, out=tmp)
    Xq = tmp.astype(np.int8)

    xrows_used = B_SCANS * S_out + B_burn
    xrows = ((xrows_used + 127) // 128) * 128
    per_core = B_SCANS * S_out
    in_maps = []
    for c in range(NC):
        lo = c * per_core - B_burn  # may be negative for core 0
        xq = np.zeros((xrows, I_SZ), np.int8)
        xs = np.zeros((xrows, 1), np.float32)
        src0 = max(lo, 0)
        src1 = min(lo + xrows, T)
        if src1 > src0:
            xq[src0 - lo:src1 - lo] = Xq[src0:src1]
            xs[src0 - lo:src1 - lo, 0] = xs_full[src0:src1]
        in_maps.append({"xq": xq, "xs": xs,
                        "wpack": blob[c * WSHARD:(c + 1) * WSHARD]})
    return in_maps


_BUILT = {}


def kernel(hidden_frames, Wc, bc, Wwg, bwg, Wwp, bwp, Wrg, brg, Wrp, brp,
           Wxh, Wrh, Whh, bh, nImg):
    assert int(nImg) == N_IMG
    S_out, B_burn = 512, 384
    key = (S_out, B_burn)
    if key not in _BUILT:
        _BUILT[key] = build(S_out=S_out, B_burn=B_burn)
    nc = _BUILT[key]
    in_maps = make_inputs_per_core(
        np.asarray(hidden_frames), np.asarray(Wc), np.asarray(bc),
        np.asarray(Wwg), np.asarray(bwg), np.asarray(Wwp), np.asarray(bwp),
        np.asarray(Wrg), np.asarray(brg), np.asarray(Wrp), np.asarray(brp),
        np.asarray(Wxh), np.asarray(Wrh), np.asarray(Whh), np.asarray(bh),
        S_out=S_out, B_burn=B_burn)
    res = bass_utils.run_bass_kernel_spmd(nc, in_maps, core_ids=list(range(NC)))
    q = np.concatenate([res.results[c]["outq"] for c in range(NC)], axis=0)
    sc = np.concatenate([res.results[c]["outsc"] for c in range(NC)], axis=0)
    return q.astype(np.float32) * sc.astype(np.float32)


# revision 6
# speedup vs baseline: 1.5049x; 1.0326x over previous
"""Trainium2 Bass kernel for nn_MemoryRamModule (scatter_memory).

Strategy: the reference is a strictly-sequential 32768-step scan with a
(mem[100,512], h[512]) carry, but the memory decays per step by (1-aw),
aw ~ softmax ~ 1/100, so carry influence dies off as e^(-0.01*B). We split
time into 64 chunks of 512 steps, run 8 independent chunk-scans per core
(batched), each with a burn-in re-deriving the carry. Scan g reads input
rows [g*512-B_burn, g*512+512), zero-padded below row 0 (zero inputs
provably keep the carry exactly zero), and emits its last 512 steps as
output rows [g*512, (g+1)*512).

Per core: phase 1 projects its X slab through all x-side weight columns
(one big matmul -> PX in DRAM); phase 2 runs the 8 scans batched, with the
per-step recurrent work done as small PE matmuls (h-projections, gated
memory read, rank-1 + decay memory update) plus DVE/ACT softmax/gate ops.

Host<->device IO is minimized (the axon tunnel at ~60MB/s dominates wall
time): X ships as int8 with per-row scales, the output ships as uint8
with per-row scales, and the tiny replicated weights ship sharded 1/8th
per core and are AllGathered on-device. Compute is fp16 with fp32 PSUM.
"""
import sys, os
sys.path.insert(0, '/opt/trn_rl_repo')
import numpy as np

import concourse.bacc as bacc
import concourse.tile as tile
from concourse import mybir
from concourse import bass_utils
from concourse.bass import ds

F32 = mybir.dt.float32
F16 = mybir.dt.float16
I8 = mybir.dt.int8
U8 = mybir.dt.uint8

I_SZ = 1024
H_SZ = 512
M_SZ = 100
N_IMG = 32768
NC = 8          # cores
B_SCANS = 8     # scans (chunks) per core

# column layout of the fused projection (1280 wide)
C_Z0, C_Z1 = 0, 512        # Whh / Wxh -> Z bank
C_C0, C_C1 = 512, 1024     # Wc -> YC bank
C_S0, C_S1 = 1024, 1280    # small bank: rp[0:100] wp[100:200] rg[200] wg[201] pad
COLS = 1280
S_RP, S_WP, S_RG, S_WG = 0, 100, 200, 201

# packed-weights blob layout, f16 rows of 1024 (AllGathered on device)
OFF_XW, N_XW = 0, 1280          # [128,8,1280]
OFF_HW, N_HW = 1280, 640        # [128,4,1280]
OFF_RW, N_RW = 1920, 256        # [128,4,512]
OFF_BIAS, N_BIAS = 2176, 2      # [1,1280] (+pad)
OFF_ID, N_ID = 2178, 16         # [128,128]
OFF_CM, N_CM = 2194, 8          # [128,8,8]
OFF_CB, N_CB = 2202, 8          # [8,8,128]
WROWS = 2216                    # padded to NC*277
WSHARD = WROWS // NC

QOUT = 254.0                    # uint8 quant full-scale


def build(S_out=512, B_burn=512, T_blk=16, unroll=False):
    """Build the per-core SPMD bass program. Returns nc."""
    assert B_burn <= S_out and B_burn % T_blk == 0 and S_out % T_blk == 0
    xrows_used = B_SCANS * S_out + B_burn
    xrows = ((xrows_used + 127) // 128) * 128

    nc = bacc.Bacc("TRN2", target_bir_lowering=False, debug=False, num_devices=NC)

    xq = nc.dram_tensor("xq", [xrows, I_SZ], I8, kind="ExternalInput")
    xs = nc.dram_tensor("xs", [xrows, 1], F32, kind="ExternalInput")
    wpack = nc.dram_tensor("wpack", [WSHARD, 1024], F16, kind="ExternalInput")
    wstage = nc.dram_tensor("wstage", [WSHARD, 1024], F16, kind="Internal")
    wfull = nc.dram_tensor("wfull", [WROWS, 1024], F16, kind="Internal")
    px = nc.dram_tensor("px", [xrows, COLS], F16, kind="Internal")
    outq_d = nc.dram_tensor("outq", [B_SCANS * S_out, H_SZ], U8, kind="ExternalOutput")
    outsc_d = nc.dram_tensor("outsc", [B_SCANS * S_out, 1], F16, kind="ExternalOutput")

    with tile.TileContext(nc) as tc:
        import contextlib
        with contextlib.ExitStack() as ctx:
            # on-device weight AllGather: each core contributes 1/NC of blob
            # (collectives can't read IO tensors, so stage through Internal)
            ld0 = nc.sync.dma_start(out=wstage.ap(), in_=wpack.ap())
            cc = nc.gpsimd.collective_compute(
                kind="AllGather", op=mybir.AluOpType.bypass,
                replica_groups=[list(range(NC))],
                ins=[wstage.ap()], outs=[wfull.ap()])
            tile.add_dep_helper(cc.ins, ld0.ins, reason="stage wpack")
            wf = wfull.ap()

            consts = ctx.enter_context(tc.tile_pool(name="consts", bufs=1))
            WH = consts.tile([128, 4, COLS], F16)
            WRH = consts.tile([128, 4, H_SZ], F16)
            BIAS = consts.tile([1, COLS], F16)
            IDENT = consts.tile([128, 128], F16)
            COLM = consts.tile([128, B_SCANS, B_SCANS], F16)
            COLMB = consts.tile([B_SCANS, B_SCANS, 128], F16)
            ONES = consts.tile([1, 128], F16)
            nc.vector.memset(ONES, 1.0)
            wloads = [
                nc.sync.dma_start(out=WH, in_=wf[OFF_HW:OFF_HW + N_HW, :]
                                  .rearrange("(p r) c -> p (r c)", r=5)
                                  .rearrange("p (a b) -> p a b", a=4)),
                nc.sync.dma_start(out=WRH, in_=wf[OFF_RW:OFF_RW + N_RW, :]
                                  .rearrange("(p r) c -> p (r c)", r=2)
                                  .rearrange("p (a b) -> p a b", a=4)),
                nc.sync.dma_start(out=BIAS[0:1, 0:1024],
                                  in_=wf[OFF_BIAS:OFF_BIAS + 1, :]),
                nc.sync.dma_start(out=BIAS[0:1, 1024:COLS],
                                  in_=wf[OFF_BIAS + 1:OFF_BIAS + 2, 0:COLS - 1024]),
                nc.sync.dma_start(out=IDENT, in_=wf[OFF_ID:OFF_ID + N_ID, :]
                                  .rearrange("r (e c) -> (r e) c", c=128)),
                nc.sync.dma_start(out=COLM, in_=wf[OFF_CM:OFF_CM + N_CM, :]
                                  .rearrange("r (e c) -> (r e) c", c=64)
                                  .rearrange("p (a b) -> p a b", a=B_SCANS)),
                nc.sync.dma_start(out=COLMB, in_=wf[OFF_CB:OFF_CB + N_CB, :]
                                  .rearrange("r (a b) -> r a b", a=B_SCANS)),
            ]
            for ld in wloads:
                tile.add_dep_helper(ld.ins, cc.ins, reason="allgather weights")

            # ---------------- phase 1: PX = X @ Wx_all + bias ----------------
            px_stores = []
            n_tchunks = xrows // 128
            with tc.tile_pool(name="p1", bufs=2) as p1, \
                 tc.tile_pool(name="p1w", bufs=1) as p1w, \
                 tc.tile_pool(name="p1ps", bufs=2, space="PSUM") as p1ps, \
                 tc.tile_pool(name="p1pst", bufs=2, space="PSUM") as p1pst:
                XW = p1w.tile([128, 8, COLS], F16)
                ldxw = nc.sync.dma_start(out=XW, in_=wf[OFF_XW:OFF_XW + N_XW, :]
                                         .rearrange("(p r) c -> p (r c)", r=10)
                                         .rearrange("p (a b) -> p a b", a=8))
                tile.add_dep_helper(ldxw.ins, cc.ins, reason="allgather weights")
                for tck in range(n_tchunks):
                    XQB = p1.tile([128, I_SZ], I8, tag="xqb")
                    XSC = p1.tile([128, 1], F32, tag="xsc")
                    nc.sync.dma_start(out=XQB, in_=xq.ap()[tck * 128:(tck + 1) * 128, :])
                    nc.sync.dma_start(out=XSC, in_=xs.ap()[tck * 128:(tck + 1) * 128, :])
                    XBLK = p1.tile([128, I_SZ], F16, tag="xblk")
                    nc.scalar.activation(XBLK, XQB,
                                         mybir.ActivationFunctionType.Copy,
                                         scale=XSC[:, 0:1])
                    XT = p1.tile([128, 8, 128], F16, tag="xt")
                    for k in range(8):
                        tp = p1pst.tile([128, 128], F16, tag="tp")
                        nc.tensor.transpose(tp, XBLK[:, k * 128:(k + 1) * 128], IDENT)
                        if k % 2 == 0:
                            nc.vector.tensor_copy(XT[:, k, :], tp)
                        else:
                            nc.scalar.copy(XT[:, k, :], tp)
                    PXB = p1.tile([128, COLS], F16, tag="pxb")
                    for (c0, c1) in ((C_Z0, C_Z1), (C_C0, C_C1), (C_S0, C_S1)):
                        ps = p1ps.tile([128, c1 - c0], F32, tag=f"ps{c0}")
                        for k in range(8):
                            nc.tensor.matmul(ps, XT[:, k, :], XW[:, k, c0:c1],
                                             start=(k == 0), stop=False)
                        nc.tensor.matmul(ps, ONES[0:1, 0:128], BIAS[0:1, c0:c1],
                                         start=False, stop=True)
                        if c0 == C_Z0:
                            nc.vector.tensor_copy(PXB[:, c0:c1], ps)
                        else:
                            nc.scalar.copy(PXB[:, c0:c1], ps)
                    st = nc.sync.dma_start(out=px.ap()[tck * 128:(tck + 1) * 128, :], in_=PXB)
                    px_stores.append(st)

            # ---------------- phase 2: batched scans ----------------
            st_pool = ctx.enter_context(tc.tile_pool(name="state", bufs=1))
            MEMC = st_pool.tile([128, B_SCANS, H_SZ], F16)    # [0:100]=mem
            ADIAG = st_pool.tile([128, B_SCANS, M_SZ], F16)   # [0:100]=diag
            HT_a = st_pool.tile([128, 4, B_SCANS], F16)
            HT_b = st_pool.tile([128, 4, B_SCANS], F16)
            PXS = st_pool.tile([B_SCANS, T_blk, COLS], F16)
            OUTS_s = st_pool.tile([B_SCANS, T_blk, H_SZ], F16)
            OUTQ_s = st_pool.tile([B_SCANS, T_blk, H_SZ], U8)
            OUTSC_s = st_pool.tile([B_SCANS, T_blk, 1], F16)
            nc.vector.memset(MEMC[0:101, :, :], 0.0)
            nc.vector.memset(HT_a[:, :, :], 0.0)

            ps_pool = ctx.enter_context(tc.tile_pool(name="ps2", bufs=1, space="PSUM"))
            Z_2 = [ps_pool.tile([B_SCANS, H_SZ], F32, tag=f"z{i}", name=f"zps{i}") for i in range(2)]
            YC_ps = ps_pool.tile([B_SCANS, H_SZ], F32, tag="yc")
            YS_ps = ps_pool.tile([B_SCANS, C_S1 - C_S0], F32, tag="ys")
            UPD_ps = [ps_pool.tile([M_SZ, H_SZ], F32, tag=f"upd{i}", name=f"updps{i}") for i in range(2)]
            MISC_ps = [ps_pool.tile([128, 1024], F16, tag=f"misc{i}", name=f"miscps{i}") for i in range(2)]

            sm_pool = ctx.enter_context(tc.tile_pool(name="small", bufs=2))

            def emit_step(s, HT_in, HT_out, OUTS, quant):
                """One scan step for all B_SCANS scans. s = slot in [0, T_blk)."""
                Z_ps = Z_2[s % 2]
                # --- YS matmuls first: they gate the whole step chain ---
                for (c0, c1, ps) in ((C_S0, C_S1, YS_ps),):
                    nc.tensor.matmul(ps, IDENT[0:B_SCANS, 0:B_SCANS],
                                     PXS[:, s, c0:c1], start=True, stop=False)
                    for k in range(4):
                        nc.tensor.matmul(ps, HT_in[:, k, :], WH[:, k, c0:c1],
                                         start=False, stop=(k == 3))
                # --- softmax(ar) first: it gates the critical read chain ---
                AR = sm_pool.tile([B_SCANS, M_SZ], F16, tag="ar")
                SMr = sm_pool.tile([B_SCANS, 1], F32, tag="smr")
                GOS = sm_pool.tile([B_SCANS, 1], F32, tag="gos")
                nc.scalar.activation(AR, YS_ps[:, S_RP:S_RP + M_SZ],
                                     mybir.ActivationFunctionType.Exp,
                                     scale=1.0, accum_out=SMr)
                nc.vector.reciprocal(SMr, SMr)
                # --- gates: go/gw via tanh (one ACT table set with Exp/Relu) ---
                TG = sm_pool.tile([B_SCANS, 2], F32, tag="tg")
                G = sm_pool.tile([B_SCANS, 2], F32, tag="g")
                nc.scalar.activation(TG, YS_ps[:, S_RG:S_WG + 1],
                                     mybir.ActivationFunctionType.Tanh, scale=0.5)
                nc.vector.tensor_scalar(G, TG, 0.5, 0.5,
                                        mybir.AluOpType.mult, mybir.AluOpType.add)
                nc.vector.tensor_scalar(GOS, G[:, 0:1], SMr[:, 0:1], None,
                                        mybir.AluOpType.mult)
                AW = sm_pool.tile([B_SCANS, M_SZ], F16, tag="aw")
                SMw = sm_pool.tile([B_SCANS, 1], F32, tag="smw")
                AWGW = sm_pool.tile([B_SCANS, M_SZ], F16, tag="awgw")
                nc.scalar.activation(AW, YS_ps[:, S_WP:S_WP + M_SZ],
                                     mybir.ActivationFunctionType.Exp,
                                     scale=1.0, accum_out=SMw)
                nc.vector.reciprocal(SMw, SMw)
                nc.vector.tensor_scalar(AW, AW, SMw[:, 0:1], None, mybir.AluOpType.mult)
                nc.vector.tensor_scalar(AWGW, AW, G[:, 1:2], None, mybir.AluOpType.mult)
                MAWGW = sm_pool.tile([B_SCANS, B_SCANS, M_SZ], F16, tag="mawgw")
                nc.vector.tensor_tensor(
                    MAWGW, AWGW.unsqueeze(1).broadcast_to((B_SCANS, B_SCANS, M_SZ)),
                    COLMB[:, :, 0:M_SZ], mybir.AluOpType.mult)
                # --- transpose ar immediately (critical); aw separately later ---
                ART = sm_pool.tile([M_SZ, B_SCANS], F16, tag="art")
                AWT = sm_pool.tile([M_SZ, B_SCANS], F16, tag="awt")
                tpa = MISC_ps[0]
                nc.tensor.transpose(tpa[0:M_SZ, 0:B_SCANS], AR, IDENT[0:B_SCANS, 0:B_SCANS])
                nc.vector.tensor_copy(ART, tpa[0:M_SZ, 0:B_SCANS])
                nc.tensor.transpose(tpa[0:M_SZ, B_SCANS:2 * B_SCANS], AW,
                                    IDENT[0:B_SCANS, 0:B_SCANS])
                nc.vector.tensor_copy(AWT, tpa[0:M_SZ, B_SCANS:2 * B_SCANS])
                # --- masked ar lhsT (one op, critical) ---
                MART = sm_pool.tile([M_SZ, B_SCANS, B_SCANS], F16, tag="mart")
                nc.vector.tensor_tensor(
                    MART, ART.unsqueeze(1).broadcast_to((M_SZ, B_SCANS, B_SCANS)),
                    COLM[0:M_SZ, :, :], mybir.AluOpType.mult)
                W1AWT = sm_pool.tile([M_SZ, B_SCANS], F16, tag="w1awt")
                nc.vector.tensor_scalar(W1AWT, AWT, -1.0, 1.0,
                                        mybir.AluOpType.mult, mybir.AluOpType.add)
                nc.vector.tensor_tensor(
                    ADIAG[0:M_SZ, :, :],
                    IDENT[0:M_SZ, 0:M_SZ].unsqueeze(1).broadcast_to((M_SZ, B_SCANS, M_SZ)),
                    W1AWT.unsqueeze(2).broadcast_to((M_SZ, B_SCANS, M_SZ)),
                    mybir.AluOpType.mult)
                # --- gated memory read: RRAW[j] = ar_j @ mem_j ---
                RR = MISC_ps[1].bitcast(F32)
                for j in range(B_SCANS):
                    nc.tensor.matmul(RR[0:B_SCANS, 0:H_SZ], MART[:, j, :],
                                     MEMC[0:M_SZ, j, :],
                                     start=(j == 0), stop=(j == B_SCANS - 1))
                R = sm_pool.tile([B_SCANS, H_SZ], F16, tag="r")
                nc.vector.tensor_scalar(R, RR[0:B_SCANS, 0:H_SZ], GOS[:, 0:1], None,
                                        mybir.AluOpType.mult)
                # --- YC and Z streams (filler priority; Z group stays open for Wrh) ---
                for (c0, c1, ps) in ((C_C0, C_C1, YC_ps), (C_Z0, C_Z1, Z_ps)):
                    nc.tensor.matmul(ps, IDENT[0:B_SCANS, 0:B_SCANS],
                                     PXS[:, s, c0:c1], start=True, stop=False)
                    last = (c0 != C_Z0)
                    for k in range(4):
                        nc.tensor.matmul(ps, HT_in[:, k, :], WH[:, k, c0:c1],
                                         start=False, stop=(last and k == 3))
                C = sm_pool.tile([B_SCANS, H_SZ], F16, tag="c")
                nc.scalar.activation(C, YC_ps, mybir.ActivationFunctionType.Relu)
                # --- R^T (4 transposes into one bank, one copy); Z += R @ Wrh ---
                RT = sm_pool.tile([128, 4, B_SCANS], F16, tag="rt")
                tpr = MISC_ps[1]
                for k in range(4):
                    nc.tensor.transpose(tpr[:, k * B_SCANS:(k + 1) * B_SCANS],
                                        R[:, k * 128:(k + 1) * 128],
                                        IDENT[0:B_SCANS, 0:B_SCANS])
                nc.vector.tensor_copy(RT, tpr[:, 0:4 * B_SCANS])
                for k in range(4):
                    nc.tensor.matmul(Z_ps, RT[:, k, :], WRH[:, k, :],
                                     start=False, stop=(k == 3))
                # --- h_new ---
                nc.scalar.activation(OUTS[:, s, :], Z_ps, mybir.ActivationFunctionType.Relu)
                # --- quantize h row to uint8 with per-row scale (output steps) ---
                if quant:
                    RMX = sm_pool.tile([B_SCANS, 1], F32, tag="rmx")
                    RSC = sm_pool.tile([B_SCANS, 1], F32, tag="rsc")
                    nc.vector.reduce_max(RMX, OUTS[:, s, :], axis=mybir.AxisListType.X)
                    nc.vector.tensor_scalar(RMX, RMX, 1.0 / QOUT, 1e-7,
                                            mybir.AluOpType.mult, mybir.AluOpType.max)
                    nc.vector.reciprocal(RSC, RMX)
                    nc.vector.tensor_scalar(OUTQ_s[:, s, :], OUTS[:, s, :],
                                            RSC[:, 0:1], None,
                                            mybir.AluOpType.mult)
                    nc.scalar.copy(OUTSC_s[:, s, 0:1], RMX)
                # --- memory update: mem = diag(1-aw) mem + awgw (x) c ---
                for j in range(B_SCANS):
                    ups = UPD_ps[j % 2]
                    nc.tensor.matmul(ups, ADIAG[0:M_SZ, j, :],
                                     MEMC[0:M_SZ, j, :], start=True, stop=False)
                    nc.tensor.matmul(ups, MAWGW[:, j, :], C,
                                     start=False, stop=True)
                    if j % 2 == 0:
                        nc.scalar.copy(MEMC[0:M_SZ, j, :], ups)
                    else:
                        nc.vector.tensor_copy(MEMC[0:M_SZ, j, :], ups)

                # --- H^T for next step (4 transposes, one copy) ---
                tph = MISC_ps[0]
                for k in range(4):
                    nc.tensor.transpose(tph[:, k * B_SCANS:(k + 1) * B_SCANS],
                                        OUTS[:, s, k * 128:(k + 1) * 128],
                                        IDENT[0:B_SCANS, 0:B_SCANS])
                nc.vector.tensor_copy(HT_out[:, :, :], tph[:, 0:4 * B_SCANS])

            pxA = px.ap()[0:B_SCANS * S_out, :].rearrange("(a t) n -> a t n", t=S_out)
            pxB = px.ap()[B_burn:B_burn + B_SCANS * S_out, :].rearrange("(a t) n -> a t n", t=S_out)
            outqv = outq_d.ap().rearrange("(j t) h -> j t h", t=S_out)
            outscv = outsc_d.ap().rearrange("(j t) h -> j t h", t=S_out)

            def body_burn(i):
                ldA = nc.sync.dma_start(out=PXS, in_=pxA[0:B_SCANS, :, :][:, ds(i, T_blk), :])
                for st in px_stores:
                    tile.add_dep_helper(ldA.ins, st.ins, reason="phase1 px ready")
                for s in range(T_blk):
                    HT_in = HT_a if s % 2 == 0 else HT_b
                    HT_out = HT_b if s % 2 == 0 else HT_a
                    emit_step(s, HT_in, HT_out, OUTS_s, quant=False)

            def body_out(i):
                ldB = nc.sync.dma_start(out=PXS, in_=pxB[:, ds(i, T_blk), :])
                for st in px_stores:
                    tile.add_dep_helper(ldB.ins, st.ins, reason="phase1 px ready")
                for s in range(T_blk):
                    HT_in = HT_a if s % 2 == 0 else HT_b
                    HT_out = HT_b if s % 2 == 0 else HT_a
                    emit_step(s, HT_in, HT_out, OUTS_s, quant=True)
                nc.sync.dma_start(out=outqv[:, ds(i, T_blk), :], in_=OUTQ_s)
                nc.sync.dma_start(out=outscv[:, ds(i, T_blk), :], in_=OUTSC_s)

            if unroll:
                for i in range(0, B_burn, T_blk):
                    body_burn(i)
                for i in range(0, S_out, T_blk):
                    body_out(i)
            else:
                hints = (mybir.EngineType.PE, mybir.EngineType.DVE,
                         mybir.EngineType.Activation, mybir.EngineType.SP)
                with tc.For_i(0, B_burn, T_blk, hint_engines=hints) as i:
                    body_burn(i)
                with tc.For_i(0, S_out, T_blk, hint_engines=hints) as i:
                    body_out(i)

    nc.compile()
    return nc


def pack_weights(Wc, bc, Wwg, bwg, Wwp, bwp, Wrg, brg, Wrp, brp,
                 Wxh, Wrh, Whh, bh):
    I, H, M = I_SZ, H_SZ, M_SZ
    Wx_all = np.zeros((I, COLS), np.float32)
    Wh_all = np.zeros((H, COLS), np.float32)
    bias_all = np.zeros((1, COLS), np.float32)
    Wx_all[:, C_Z0:C_Z1] = Wxh
    Wh_all[:, C_Z0:C_Z1] = Whh
    Wx_all[:, C_C0:C_C1] = Wc[:I]
    Wh_all[:, C_C0:C_C1] = Wc[I:]
    Wx_all[:, C_S0 + S_RP:C_S0 + S_RP + M] = Wrp[:I]
    Wh_all[:, C_S0 + S_RP:C_S0 + S_RP + M] = Wrp[I:]
    Wx_all[:, C_S0 + S_WP:C_S0 + S_WP + M] = Wwp[:I]
    Wh_all[:, C_S0 + S_WP:C_S0 + S_WP + M] = Wwp[I:]
    Wx_all[:, C_S0 + S_RG] = Wrg[:I, 0]
    Wh_all[:, C_S0 + S_RG] = Wrg[I:, 0]
    Wx_all[:, C_S0 + S_WG] = Wwg[:I, 0]
    Wh_all[:, C_S0 + S_WG] = Wwg[I:, 0]
    bias_all[0, C_Z0:C_Z1] = bh
    bias_all[0, C_C0:C_C1] = bc
    bias_all[0, C_S0 + S_RP:C_S0 + S_RP + M] = brp
    bias_all[0, C_S0 + S_WP:C_S0 + S_WP + M] = bwp
    bias_all[0, C_S0 + S_RG] = np.float32(np.asarray(brg).reshape(-1)[0])
    bias_all[0, C_S0 + S_WG] = np.float32(np.asarray(bwg).reshape(-1)[0])

    f16 = np.float16
    xw = np.ascontiguousarray(
        Wx_all.reshape(8, 128, COLS).transpose(1, 0, 2)).astype(f16)
    hww = np.ascontiguousarray(
        Wh_all.reshape(4, 128, COLS).transpose(1, 0, 2)).astype(f16)
    rww = np.ascontiguousarray(
        Wrh.astype(np.float32).reshape(4, 128, H).transpose(1, 0, 2)).astype(f16)
    ident = np.eye(128, dtype=f16)
    colm = np.zeros((128, B_SCANS, B_SCANS), f16)
    for j in range(B_SCANS):
        colm[:, j, j] = 1.0
    colmb = np.zeros((B_SCANS, B_SCANS, 128), f16)
    for j in range(B_SCANS):
        colmb[j, j, :] = 1.0

    blob = np.zeros((WROWS, 1024), f16)
    blob[OFF_XW:OFF_XW + N_XW] = xw.reshape(N_XW, 1024)
    blob[OFF_HW:OFF_HW + N_HW] = hww.reshape(N_HW, 1024)
    blob[OFF_RW:OFF_RW + N_RW] = rww.reshape(N_RW, 1024)
    bias16 = bias_all.astype(f16).reshape(-1)
    blob[OFF_BIAS, :1024] = bias16[:1024]
    blob[OFF_BIAS + 1, :COLS - 1024] = bias16[1024:]
    blob[OFF_ID:OFF_ID + N_ID] = ident.reshape(N_ID, 1024)
    blob[OFF_CM:OFF_CM + N_CM] = colm.reshape(N_CM, 1024)
    blob[OFF_CB:OFF_CB + N_CB] = colmb.reshape(N_CB, 1024)
    return blob


def make_inputs_per_core(hidden_frames, Wc, bc, Wwg, bwg, Wwp, bwp, Wrg, brg,
                         Wrp, brp, Wxh, Wrh, Whh, bh, S_out=512, B_burn=512):
    blob = pack_weights(Wc, bc, Wwg, bwg, Wwp, bwp, Wrg, brg, Wrp, brp,
                        Wxh, Wrh, Whh, bh)

    X = np.asarray(hidden_frames, np.float32)
    T = X.shape[0]
    amax = np.abs(X).max(axis=1)
    xs_full = (np.maximum(amax, 1e-9) / 127.0).astype(np.float32)
    tmp = X * (np.float32(1.0) / xs_full)[:, None]
    np.rint(tmp, out=tmp)
    Xq = tmp.astype(np.int8)

    xrows_used = B_SCANS * S_out + B_burn
    xrows = ((xrows_used + 127) // 128) * 128
    per_core = B_SCANS * S_out
    in_maps = []
    for c in range(NC):
        lo = c * per_core - B_burn  # may be negative for core 0
        xq = np.zeros((xrows, I_SZ), np.int8)
        xs = np.zeros((xrows, 1), np.float32)
        src0 = max(lo, 0)
        src1 = min(lo + xrows, T)
        if src1 > src0:
            xq[src0 - lo:src1 - lo] = Xq[src0:src1]
            xs[src0 - lo:src1 - lo, 0] = xs_full[src0:src1]
        in_maps.append({"xq": xq, "xs": xs,
                        "wpack": blob[c * WSHARD:(c + 1) * WSHARD]})
    return in_maps


_BUILT = {}


def kernel(hidden_frames, Wc, bc, Wwg, bwg, Wwp, bwp, Wrg, brg, Wrp, brp,
           Wxh, Wrh, Whh, bh, nImg):
    assert int(nImg) == N_IMG
    S_out, B_burn = 512, 384
    key = (S_out, B_burn)
    if key not in _BUILT:
        _BUILT[key] = build(S_out=S_out, B_burn=B_burn)
    nc = _BUILT[key]
    in_maps = make_inputs_per_core(
        np.asarray(hidden_frames), np.asarray(Wc), np.asarray(bc),
        np.asarray(Wwg), np.asarray(bwg), np.asarray(Wwp), np.asarray(bwp),
        np.asarray(Wrg), np.asarray(brg), np.asarray(Wrp), np.asarray(brp),
        np.asarray(Wxh), np.asarray(Wrh), np.asarray(Whh), np.asarray(bh),
        S_out=S_out, B_burn=B_burn)
    res = bass_utils.run_bass_kernel_spmd(nc, in_maps, core_ids=list(range(NC)))
    q = np.concatenate([res.results[c]["outq"] for c in range(NC)], axis=0)
    sc = np.concatenate([res.results[c]["outsc"] for c in range(NC)], axis=0)
    return q.astype(np.float32) * sc.astype(np.float32)


# revision 7
# speedup vs baseline: 1.8794x; 1.2489x over previous
"""Trainium2 Bass kernel for nn_MemoryRamModule (scatter_memory).

Strategy: the reference is a strictly-sequential 32768-step scan with a
(mem[100,512], h[512]) carry, but the memory decays per step by (1-aw),
aw ~ softmax ~ 1/100, so carry influence dies off as e^(-0.01*B). We split
time into 64 chunks of 512 steps, run 8 independent chunk-scans per core
(batched), each with a burn-in re-deriving the carry. Scan g reads input
rows [g*512-B_burn, g*512+512), zero-padded below row 0 (zero inputs
provably keep the carry exactly zero), and emits its last 512 steps as
output rows [g*512, (g+1)*512).

Per core: phase 1 projects its X slab through all x-side weight columns
(one big matmul -> PX in DRAM); phase 2 runs the 8 scans batched, with the
per-step recurrent work done as small PE matmuls (h-projections, gated
memory read, rank-1 + decay memory update) plus DVE/ACT softmax/gate ops.

Host<->device IO is minimized (the axon tunnel at ~60MB/s dominates wall
time): ONE packed input tensor per core (X as int8 + per-row f16 scales +
a 1/8 shard of the f16 weights, AllGathered on device) and ONE packed
output tensor (h as uint8 + per-row f16 scales). Compute is fp16 with
fp32 PSUM.
"""
import sys, os
sys.path.insert(0, '/opt/trn_rl_repo')
import numpy as np

import concourse.bacc as bacc
import concourse.tile as tile
from concourse import mybir
from concourse import bass_utils
from concourse.bass import ds

F32 = mybir.dt.float32
F16 = mybir.dt.float16
I8 = mybir.dt.int8
U8 = mybir.dt.uint8

I_SZ = 1024
H_SZ = 512
M_SZ = 100
N_IMG = 32768
NC = 8          # cores
B_SCANS = 8     # scans (chunks) per core

# column layout of the fused projection (1280 wide)
C_Z0, C_Z1 = 0, 512        # Whh / Wxh -> Z bank
C_C0, C_C1 = 512, 1024     # Wc -> YC bank
C_S0, C_S1 = 1024, 1280    # small bank: rp[0:100] wp[100:200] rg[200] wg[201] pad
COLS = 1280
S_RP, S_WP, S_RG, S_WG = 0, 100, 200, 201

# packed-weights blob layout, f16 rows of 1024 (AllGathered on device)
OFF_XW, N_XW = 0, 1280          # [128,8,1280]
OFF_HW, N_HW = 1280, 640        # [128,4,1280]
OFF_RW, N_RW = 1920, 256        # [128,4,512]
OFF_BIAS, N_BIAS = 2176, 2      # [1,1280] (+pad)
OFF_ID, N_ID = 2178, 16         # [128,128]
OFF_CM, N_CM = 2194, 8          # [128,8,8]
OFF_CB, N_CB = 2202, 8          # [8,8,128]
WROWS = 2216                    # padded to NC*277
WSHARD = WROWS // NC

QOUT = 254.0                    # uint8 quant full-scale


def _xrows(S_out, B_burn):
    return ((B_SCANS * S_out + B_burn + 127) // 128) * 128


def _in_layout(S_out, B_burn):
    """Packed input tensor layout, in f16 rows of 1024 (2048 bytes)."""
    xrows = _xrows(S_out, B_burn)
    r_xq = xrows // 2                       # int8 [xrows,1024] = xrows/2 rows
    r_xs = (xrows * 2 + 2047) // 2048       # f16 [xrows] scales
    return r_xq, r_xs, r_xq + r_xs + WSHARD  # xq rows, xs rows, total rows


def build(S_out=512, B_burn=512, T_blk=16, unroll=False):
    """Build the per-core SPMD bass program. Returns nc."""
    assert B_burn <= S_out and B_burn % T_blk == 0 and S_out % T_blk == 0
    xrows = _xrows(S_out, B_burn)
    R_XQ, R_XS, R_IN = _in_layout(S_out, B_burn)
    assert S_out % 4 == 0 and (B_SCANS * S_out) % 2048 == 0
    r_oq = B_SCANS * S_out // 4             # uint8 h rows, 4 per packed row
    r_osc = B_SCANS * S_out * 2 // 2048     # f16 scales
    R_OUT = r_oq + r_osc

    nc = bacc.Bacc("TRN2", target_bir_lowering=False, debug=False, num_devices=NC)

    xin = nc.dram_tensor("xin", [R_IN, 1024], F16, kind="ExternalInput")
    wstage = nc.dram_tensor("wstage", [WSHARD, 1024], F16, kind="Internal")
    wfull = nc.dram_tensor("wfull", [WROWS, 1024], F16, kind="Internal")
    px = nc.dram_tensor("px", [xrows, COLS], F16, kind="Internal")
    outp_d = nc.dram_tensor("outp", [R_OUT, 1024], F16, kind="ExternalOutput")

    xq_v = xin.ap()[0:R_XQ, :].bitcast(I8).rearrange("r (a c) -> (r a) c", c=I_SZ)
    xs_v = xin.ap()[R_XQ:R_XQ + R_XS, :].rearrange("r (p c) -> (r p) c", c=1)

    with tile.TileContext(nc) as tc:
        import contextlib
        with contextlib.ExitStack() as ctx:
            # on-device weight AllGather: each core contributes 1/NC of blob
            # (collectives can't read IO tensors, so stage through Internal)
            ld0 = nc.sync.dma_start(out=wstage.ap(),
                                    in_=xin.ap()[R_XQ + R_XS:R_IN, :])
            cc = nc.gpsimd.collective_compute(
                kind="AllGather", op=mybir.AluOpType.bypass,
                replica_groups=[list(range(NC))],
                ins=[wstage.ap()], outs=[wfull.ap()])
            tile.add_dep_helper(cc.ins, ld0.ins, reason="stage wpack")
            wf = wfull.ap()

            consts = ctx.enter_context(tc.tile_pool(name="consts", bufs=1))
            WH = consts.tile([128, 4, COLS], F16)
            WRH = consts.tile([128, 4, H_SZ], F16)
            BIAS = consts.tile([1, COLS], F16)
            IDENT = consts.tile([128, 128], F16)
            COLM = consts.tile([128, B_SCANS, B_SCANS], F16)
            COLMB = consts.tile([B_SCANS, B_SCANS, 128], F16)
            ONES = consts.tile([1, 128], F16)
            nc.vector.memset(ONES, 1.0)
            wloads = [
                nc.sync.dma_start(out=WH, in_=wf[OFF_HW:OFF_HW + N_HW, :]
                                  .rearrange("(p r) c -> p (r c)", r=5)
                                  .rearrange("p (a b) -> p a b", a=4)),
                nc.sync.dma_start(out=WRH, in_=wf[OFF_RW:OFF_RW + N_RW, :]
                                  .rearrange("(p r) c -> p (r c)", r=2)
                                  .rearrange("p (a b) -> p a b", a=4)),
                nc.sync.dma_start(out=BIAS[0:1, 0:1024],
                                  in_=wf[OFF_BIAS:OFF_BIAS + 1, :]),
                nc.sync.dma_start(out=BIAS[0:1, 1024:COLS],
                                  in_=wf[OFF_BIAS + 1:OFF_BIAS + 2, 0:COLS - 1024]),
                nc.sync.dma_start(out=IDENT, in_=wf[OFF_ID:OFF_ID + N_ID, :]
                                  .rearrange("r (e c) -> (r e) c", c=128)),
                nc.sync.dma_start(out=COLM, in_=wf[OFF_CM:OFF_CM + N_CM, :]
                                  .rearrange("r (e c) -> (r e) c", c=64)
                                  .rearrange("p (a b) -> p a b", a=B_SCANS)),
                nc.sync.dma_start(out=COLMB, in_=wf[OFF_CB:OFF_CB + N_CB, :]
                                  .rearrange("r (a b) -> r a b", a=B_SCANS)),
            ]
            for ld in wloads:
                tile.add_dep_helper(ld.ins, cc.ins, reason="allgather weights")

            # ---------------- phase 1: PX = X @ Wx_all + bias ----------------
            px_stores = []
            n_tchunks = xrows // 128
            with tc.tile_pool(name="p1", bufs=2) as p1, \
                 tc.tile_pool(name="p1w", bufs=1) as p1w, \
                 tc.tile_pool(name="p1ps", bufs=2, space="PSUM") as p1ps, \
                 tc.tile_pool(name="p1pst", bufs=2, space="PSUM") as p1pst:
                XW = p1w.tile([128, 8, COLS], F16)
                ldxw = nc.sync.dma_start(out=XW, in_=wf[OFF_XW:OFF_XW + N_XW, :]
                                         .rearrange("(p r) c -> p (r c)", r=10)
                                         .rearrange("p (a b) -> p a b", a=8))
                tile.add_dep_helper(ldxw.ins, cc.ins, reason="allgather weights")
                for tck in range(n_tchunks):
                    XQB = p1.tile([128, I_SZ], I8, tag="xqb")
                    XS16 = p1.tile([128, 1], F16, tag="xs16")
                    XSC = p1.tile([128, 1], F32, tag="xsc")
                    nc.sync.dma_start(out=XQB, in_=xq_v[tck * 128:(tck + 1) * 128, :])
                    nc.sync.dma_start(out=XS16, in_=xs_v[tck * 128:(tck + 1) * 128, :])
                    nc.vector.tensor_copy(XSC, XS16)
                    XBLK = p1.tile([128, I_SZ], F16, tag="xblk")
                    nc.scalar.activation(XBLK, XQB,
                                         mybir.ActivationFunctionType.Copy,
                                         scale=XSC[:, 0:1])
                    XT = p1.tile([128, 8, 128], F16, tag="xt")
                    for k in range(8):
                        tp = p1pst.tile([128, 128], F16, tag="tp")
                        nc.tensor.transpose(tp, XBLK[:, k * 128:(k + 1) * 128], IDENT)
                        if k % 2 == 0:
                            nc.vector.tensor_copy(XT[:, k, :], tp)
                        else:
                            nc.scalar.copy(XT[:, k, :], tp)
                    PXB = p1.tile([128, COLS], F16, tag="pxb")
                    for (c0, c1) in ((C_Z0, C_Z1), (C_C0, C_C1), (C_S0, C_S1)):
                        ps = p1ps.tile([128, c1 - c0], F32, tag=f"ps{c0}")
                        for k in range(8):
                            nc.tensor.matmul(ps, XT[:, k, :], XW[:, k, c0:c1],
                                             start=(k == 0), stop=False)
                        nc.tensor.matmul(ps, ONES[0:1, 0:128], BIAS[0:1, c0:c1],
                                         start=False, stop=True)
                        if c0 == C_Z0:
                            nc.vector.tensor_copy(PXB[:, c0:c1], ps)
                        else:
                            nc.scalar.copy(PXB[:, c0:c1], ps)
                    st = nc.sync.dma_start(out=px.ap()[tck * 128:(tck + 1) * 128, :], in_=PXB)
                    px_stores.append(st)

            # ---------------- phase 2: batched scans ----------------
            st_pool = ctx.enter_context(tc.tile_pool(name="state", bufs=1))
            MEMC = st_pool.tile([128, B_SCANS, H_SZ], F16)    # [0:100]=mem
            ADIAG = st_pool.tile([128, B_SCANS, M_SZ], F16)   # [0:100]=diag
            HT_a = st_pool.tile([128, 4, B_SCANS], F16)
            HT_b = st_pool.tile([128, 4, B_SCANS], F16)
            PXS = st_pool.tile([B_SCANS, T_blk, COLS], F16)
            OUTS_s = st_pool.tile([B_SCANS, T_blk, H_SZ], F16)
            OUTQ_s = st_pool.tile([B_SCANS, T_blk, H_SZ], U8)
            OUTSC_s = st_pool.tile([B_SCANS, T_blk], F16)
            nc.vector.memset(MEMC[0:101, :, :], 0.0)
            nc.vector.memset(HT_a[:, :, :], 0.0)

            ps_pool = ctx.enter_context(tc.tile_pool(name="ps2", bufs=1, space="PSUM"))
            Z_2 = [ps_pool.tile([B_SCANS, H_SZ], F32, tag=f"z{i}", name=f"zps{i}") for i in range(2)]
            YC_ps = ps_pool.tile([B_SCANS, H_SZ], F32, tag="yc")
            YS_ps = ps_pool.tile([B_SCANS, C_S1 - C_S0], F32, tag="ys")
            UPD_ps = [ps_pool.tile([M_SZ, H_SZ], F32, tag=f"upd{i}", name=f"updps{i}") for i in range(2)]
            MISC_ps = [ps_pool.tile([128, 1024], F16, tag=f"misc{i}", name=f"miscps{i}") for i in range(2)]

            sm_pool = ctx.enter_context(tc.tile_pool(name="small", bufs=2))

            def emit_step(s, HT_in, HT_out, OUTS, quant):
                """One scan step for all B_SCANS scans. s = slot in [0, T_blk)."""
                Z_ps = Z_2[s % 2]
                # --- YS matmuls first: they gate the whole step chain ---
                for (c0, c1, ps) in ((C_S0, C_S1, YS_ps),):
                    nc.tensor.matmul(ps, IDENT[0:B_SCANS, 0:B_SCANS],
                                     PXS[:, s, c0:c1], start=True, stop=False)
                    for k in range(4):
                        nc.tensor.matmul(ps, HT_in[:, k, :], WH[:, k, c0:c1],
                                         start=False, stop=(k == 3))
                # --- softmax(ar) first: it gates the critical read chain ---
                AR = sm_pool.tile([B_SCANS, M_SZ], F16, tag="ar")
                SMr = sm_pool.tile([B_SCANS, 1], F32, tag="smr")
                GOS = sm_pool.tile([B_SCANS, 1], F32, tag="gos")
                nc.scalar.activation(AR, YS_ps[:, S_RP:S_RP + M_SZ],
                                     mybir.ActivationFunctionType.Exp,
                                     scale=1.0, accum_out=SMr)
                nc.vector.reciprocal(SMr, SMr)
                # --- gates: go/gw via tanh (one ACT table set with Exp/Relu) ---
                TG = sm_pool.tile([B_SCANS, 2], F32, tag="tg")
                G = sm_pool.tile([B_SCANS, 2], F32, tag="g")
                nc.scalar.activation(TG, YS_ps[:, S_RG:S_WG + 1],
                                     mybir.ActivationFunctionType.Tanh, scale=0.5)
                nc.vector.tensor_scalar(G, TG, 0.5, 0.5,
                                        mybir.AluOpType.mult, mybir.AluOpType.add)
                nc.vector.tensor_scalar(GOS, G[:, 0:1], SMr[:, 0:1], None,
                                        mybir.AluOpType.mult)
                AW = sm_pool.tile([B_SCANS, M_SZ], F16, tag="aw")
                SMw = sm_pool.tile([B_SCANS, 1], F32, tag="smw")
                AWGW = sm_pool.tile([B_SCANS, M_SZ], F16, tag="awgw")
                nc.scalar.activation(AW, YS_ps[:, S_WP:S_WP + M_SZ],
                                     mybir.ActivationFunctionType.Exp,
                                     scale=1.0, accum_out=SMw)
                nc.vector.reciprocal(SMw, SMw)
                nc.vector.tensor_scalar(AW, AW, SMw[:, 0:1], None, mybir.AluOpType.mult)
                nc.vector.tensor_scalar(AWGW, AW, G[:, 1:2], None, mybir.AluOpType.mult)
                MAWGW = sm_pool.tile([B_SCANS, B_SCANS, M_SZ], F16, tag="mawgw")
                nc.vector.tensor_tensor(
                    MAWGW, AWGW.unsqueeze(1).broadcast_to((B_SCANS, B_SCANS, M_SZ)),
                    COLMB[:, :, 0:M_SZ], mybir.AluOpType.mult)
                # --- transpose ar immediately (critical); aw separately later ---
                ART = sm_pool.tile([M_SZ, B_SCANS], F16, tag="art")
                AWT = sm_pool.tile([M_SZ, B_SCANS], F16, tag="awt")
                tpa = MISC_ps[0]
                nc.tensor.transpose(tpa[0:M_SZ, 0:B_SCANS], AR, IDENT[0:B_SCANS, 0:B_SCANS])
                nc.vector.tensor_copy(ART, tpa[0:M_SZ, 0:B_SCANS])
                nc.tensor.transpose(tpa[0:M_SZ, B_SCANS:2 * B_SCANS], AW,
                                    IDENT[0:B_SCANS, 0:B_SCANS])
                nc.vector.tensor_copy(AWT, tpa[0:M_SZ, B_SCANS:2 * B_SCANS])
                # --- masked ar lhsT (one op, critical) ---
                MART = sm_pool.tile([M_SZ, B_SCANS, B_SCANS], F16, tag="mart")
                nc.vector.tensor_tensor(
                    MART, ART.unsqueeze(1).broadcast_to((M_SZ, B_SCANS, B_SCANS)),
                    COLM[0:M_SZ, :, :], mybir.AluOpType.mult)
                W1AWT = sm_pool.tile([M_SZ, B_SCANS], F16, tag="w1awt")
                nc.vector.tensor_scalar(W1AWT, AWT, -1.0, 1.0,
                                        mybir.AluOpType.mult, mybir.AluOpType.add)
                nc.vector.tensor_tensor(
                    ADIAG[0:M_SZ, :, :],
                    IDENT[0:M_SZ, 0:M_SZ].unsqueeze(1).broadcast_to((M_SZ, B_SCANS, M_SZ)),
                    W1AWT.unsqueeze(2).broadcast_to((M_SZ, B_SCANS, M_SZ)),
                    mybir.AluOpType.mult)
                # --- gated memory read: RRAW[j] = ar_j @ mem_j ---
                RR = MISC_ps[1].bitcast(F32)
                for j in range(B_SCANS):
                    nc.tensor.matmul(RR[0:B_SCANS, 0:H_SZ], MART[:, j, :],
                                     MEMC[0:M_SZ, j, :],
                                     start=(j == 0), stop=(j == B_SCANS - 1))
                R = sm_pool.tile([B_SCANS, H_SZ], F16, tag="r")
                nc.vector.tensor_scalar(R, RR[0:B_SCANS, 0:H_SZ], GOS[:, 0:1], None,
                                        mybir.AluOpType.mult)
                # --- YC and Z streams (filler priority; Z group stays open for Wrh) ---
                for (c0, c1, ps) in ((C_C0, C_C1, YC_ps), (C_Z0, C_Z1, Z_ps)):
                    nc.tensor.matmul(ps, IDENT[0:B_SCANS, 0:B_SCANS],
                                     PXS[:, s, c0:c1], start=True, stop=False)
                    last = (c0 != C_Z0)
                    for k in range(4):
                        nc.tensor.matmul(ps, HT_in[:, k, :], WH[:, k, c0:c1],
                                         start=False, stop=(last and k == 3))
                C = sm_pool.tile([B_SCANS, H_SZ], F16, tag="c")
                nc.scalar.activation(C, YC_ps, mybir.ActivationFunctionType.Relu)
                # --- R^T (4 transposes into one bank, one copy); Z += R @ Wrh ---
                RT = sm_pool.tile([128, 4, B_SCANS], F16, tag="rt")
                tpr = MISC_ps[1]
                for k in range(4):
                    nc.tensor.transpose(tpr[:, k * B_SCANS:(k + 1) * B_SCANS],
                                        R[:, k * 128:(k + 1) * 128],
                                        IDENT[0:B_SCANS, 0:B_SCANS])
                nc.vector.tensor_copy(RT, tpr[:, 0:4 * B_SCANS])
                for k in range(4):
                    nc.tensor.matmul(Z_ps, RT[:, k, :], WRH[:, k, :],
                                     start=False, stop=(k == 3))
                # --- h_new ---
                nc.scalar.activation(OUTS[:, s, :], Z_ps, mybir.ActivationFunctionType.Relu)
                # --- quantize h row to uint8 with per-row scale (output steps) ---
                if quant:
                    RMX = sm_pool.tile([B_SCANS, 1], F32, tag="rmx")
                    RSC = sm_pool.tile([B_SCANS, 1], F32, tag="rsc")
                    nc.vector.reduce_max(RMX, OUTS[:, s, :], axis=mybir.AxisListType.X)
                    nc.vector.tensor_scalar(RMX, RMX, 1.0 / QOUT, 1e-7,
                                            mybir.AluOpType.mult, mybir.AluOpType.max)
                    nc.vector.reciprocal(RSC, RMX)
                    nc.vector.tensor_scalar(OUTQ_s[:, s, :], OUTS[:, s, :],
                                            RSC[:, 0:1], None,
                                            mybir.AluOpType.mult)
                    nc.scalar.copy(OUTSC_s[:, s:s + 1], RMX)
                # --- memory update: mem = diag(1-aw) mem + awgw (x) c ---
                for j in range(B_SCANS):
                    ups = UPD_ps[j % 2]
                    nc.tensor.matmul(ups, ADIAG[0:M_SZ, j, :],
                                     MEMC[0:M_SZ, j, :], start=True, stop=False)
                    nc.tensor.matmul(ups, MAWGW[:, j, :], C,
                                     start=False, stop=True)
                    if j % 2 == 0:
                        nc.scalar.copy(MEMC[0:M_SZ, j, :], ups)
                    else:
                        nc.vector.tensor_copy(MEMC[0:M_SZ, j, :], ups)

                # --- H^T for next step (4 transposes, one copy) ---
                tph = MISC_ps[0]
                for k in range(4):
                    nc.tensor.transpose(tph[:, k * B_SCANS:(k + 1) * B_SCANS],
                                        OUTS[:, s, k * 128:(k + 1) * 128],
                                        IDENT[0:B_SCANS, 0:B_SCANS])
                nc.vector.tensor_copy(HT_out[:, :, :], tph[:, 0:4 * B_SCANS])

            pxA = px.ap()[0:B_SCANS * S_out, :].rearrange("(a t) n -> a t n", t=S_out)
            pxB = px.ap()[B_burn:B_burn + B_SCANS * S_out, :].rearrange("(a t) n -> a t n", t=S_out)
            # packed output views: q bytes then f16 scales
            outqv = outp_d.ap()[0:r_oq, :].bitcast(U8) \
                .rearrange("(j r) (f c) -> j (r f) c", j=B_SCANS, c=H_SZ)
            outscv = outp_d.ap()[r_oq:R_OUT, :] \
                .rearrange("r (j c) -> (r j) c", c=S_out)

            def body_burn(i):
                ldA = nc.sync.dma_start(out=PXS, in_=pxA[0:B_SCANS, :, :][:, ds(i, T_blk), :])
                for st in px_stores:
                    tile.add_dep_helper(ldA.ins, st.ins, reason="phase1 px ready")
                for s in range(T_blk):
                    HT_in = HT_a if s % 2 == 0 else HT_b
                    HT_out = HT_b if s % 2 == 0 else HT_a
                    emit_step(s, HT_in, HT_out, OUTS_s, quant=False)

            def body_out(i):
                ldB = nc.sync.dma_start(out=PXS, in_=pxB[:, ds(i, T_blk), :])
                for st in px_stores:
                    tile.add_dep_helper(ldB.ins, st.ins, reason="phase1 px ready")
                for s in range(T_blk):
                    HT_in = HT_a if s % 2 == 0 else HT_b
                    HT_out = HT_b if s % 2 == 0 else HT_a
                    emit_step(s, HT_in, HT_out, OUTS_s, quant=True)
                nc.sync.dma_start(out=outqv[:, ds(i, T_blk), :], in_=OUTQ_s)
                nc.sync.dma_start(out=outscv[:, ds(i, T_blk)], in_=OUTSC_s)

            if unroll:
                for i in range(0, B_burn, T_blk):
                    body_burn(i)
                for i in range(0, S_out, T_blk):
                    body_out(i)
            else:
                hints = (mybir.EngineType.PE, mybir.EngineType.DVE,
                         mybir.EngineType.Activation, mybir.EngineType.SP)
                with tc.For_i(0, B_burn, T_blk, hint_engines=hints) as i:
                    body_burn(i)
                with tc.For_i(0, S_out, T_blk, hint_engines=hints) as i:
                    body_out(i)

    nc.compile()
    return nc


def pack_weights(Wc, bc, Wwg, bwg, Wwp, bwp, Wrg, brg, Wrp, brp,
                 Wxh, Wrh, Whh, bh):
    I, H, M = I_SZ, H_SZ, M_SZ
    Wx_all = np.zeros((I, COLS), np.float32)
    Wh_all = np.zeros((H, COLS), np.float32)
    bias_all = np.zeros((1, COLS), np.float32)
    Wx_all[:, C_Z0:C_Z1] = Wxh
    Wh_all[:, C_Z0:C_Z1] = Whh
    Wx_all[:, C_C0:C_C1] = Wc[:I]
    Wh_all[:, C_C0:C_C1] = Wc[I:]
    Wx_all[:, C_S0 + S_RP:C_S0 + S_RP + M] = Wrp[:I]
    Wh_all[:, C_S0 + S_RP:C_S0 + S_RP + M] = Wrp[I:]
    Wx_all[:, C_S0 + S_WP:C_S0 + S_WP + M] = Wwp[:I]
    Wh_all[:, C_S0 + S_WP:C_S0 + S_WP + M] = Wwp[I:]
    Wx_all[:, C_S0 + S_RG] = Wrg[:I, 0]
    Wh_all[:, C_S0 + S_RG] = Wrg[I:, 0]
    Wx_all[:, C_S0 + S_WG] = Wwg[:I, 0]
    Wh_all[:, C_S0 + S_WG] = Wwg[I:, 0]
    bias_all[0, C_Z0:C_Z1] = bh
    bias_all[0, C_C0:C_C1] = bc
    bias_all[0, C_S0 + S_RP:C_S0 + S_RP + M] = brp
    bias_all[0, C_S0 + S_WP:C_S0 + S_WP + M] = bwp
    bias_all[0, C_S0 + S_RG] = np.float32(np.asarray(brg).reshape(-1)[0])
    bias_all[0, C_S0 + S_WG] = np.float32(np.asarray(bwg).reshape(-1)[0])

    f16 = np.float16
    xw = np.ascontiguousarray(
        Wx_all.reshape(8, 128, COLS).transpose(1, 0, 2)).astype(f16)
    hww = np.ascontiguousarray(
        Wh_all.reshape(4, 128, COLS).transpose(1, 0, 2)).astype(f16)
    rww = np.ascontiguousarray(
        Wrh.astype(np.float32).reshape(4, 128, H).transpose(1, 0, 2)).astype(f16)
    ident = np.eye(128, dtype=f16)
    colm = np.zeros((128, B_SCANS, B_SCANS), f16)
    for j in range(B_SCANS):
        colm[:, j, j] = 1.0
    colmb = np.zeros((B_SCANS, B_SCANS, 128), f16)
    for j in range(B_SCANS):
        colmb[j, j, :] = 1.0

    blob = np.zeros((WROWS, 1024), f16)
    blob[OFF_XW:OFF_XW + N_XW] = xw.reshape(N_XW, 1024)
    blob[OFF_HW:OFF_HW + N_HW] = hww.reshape(N_HW, 1024)
    blob[OFF_RW:OFF_RW + N_RW] = rww.reshape(N_RW, 1024)
    bias16 = bias_all.astype(f16).reshape(-1)
    blob[OFF_BIAS, :1024] = bias16[:1024]
    blob[OFF_BIAS + 1, :COLS - 1024] = bias16[1024:]
    blob[OFF_ID:OFF_ID + N_ID] = ident.reshape(N_ID, 1024)
    blob[OFF_CM:OFF_CM + N_CM] = colm.reshape(N_CM, 1024)
    blob[OFF_CB:OFF_CB + N_CB] = colmb.reshape(N_CB, 1024)
    return blob


def make_inputs_per_core(hidden_frames, Wc, bc, Wwg, bwg, Wwp, bwp, Wrg, brg,
                         Wrp, brp, Wxh, Wrh, Whh, bh, S_out=512, B_burn=512):
    blob = pack_weights(Wc, bc, Wwg, bwg, Wwp, bwp, Wrg, brg, Wrp, brp,
                        Wxh, Wrh, Whh, bh)

    X = np.asarray(hidden_frames, np.float32)
    T = X.shape[0]
    amax = np.abs(X).max(axis=1)
    xs_full = (np.maximum(amax, 1e-9) / 127.0).astype(np.float16)
    tmp = X * (np.float32(1.0) / xs_full.astype(np.float32))[:, None]
    np.rint(tmp, out=tmp)
    Xq = tmp.astype(np.int8)

    xrows = _xrows(S_out, B_burn)
    R_XQ, R_XS, R_IN = _in_layout(S_out, B_burn)
    per_core = B_SCANS * S_out
    in_maps = []
    for c in range(NC):
        lo = c * per_core - B_burn  # may be negative for core 0
        xin = np.zeros((R_IN, 1024), np.float16)
        xb = xin.view(np.uint8).reshape(R_IN, 2048)
        src0 = max(lo, 0)
        src1 = min(lo + xrows, T)
        n = src1 - src0
        if n > 0:
            d0 = src0 - lo
            xb[:R_XQ].reshape(xrows, I_SZ).view(np.int8)[d0:d0 + n] = Xq[src0:src1]
            xb[R_XQ:R_XQ + R_XS].reshape(-1).view(np.float16)[d0:d0 + n] = \
                xs_full[src0:src1]
        xb[R_XQ + R_XS:R_IN] = blob[c * WSHARD:(c + 1) * WSHARD].view(np.uint8)
        in_maps.append({"xin": xin})
    return in_maps


def unpack_results(res, S_out=512):
    """res -> full [N_IMG, H_SZ] fp32 output."""
    r_oq = B_SCANS * S_out // 4
    R_OUT = r_oq + B_SCANS * S_out * 2 // 2048
    parts = []
    for c in range(NC):
        packed = np.asarray(res.results[c]["outp"])
        xb = packed.view(np.uint8).reshape(R_OUT, 2048)
        q = xb[:r_oq].reshape(B_SCANS, S_out, H_SZ)
        sc = xb[r_oq:].reshape(-1).view(np.float16).reshape(B_SCANS, S_out)
        parts.append(q.astype(np.float32) *
                     sc.astype(np.float32)[:, :, None])
    return np.concatenate(parts, axis=0).reshape(N_IMG, H_SZ)


_BUILT = {}


def kernel(hidden_frames, Wc, bc, Wwg, bwg, Wwp, bwp, Wrg, brg, Wrp, brp,
           Wxh, Wrh, Whh, bh, nImg):
    assert int(nImg) == N_IMG
    S_out, B_burn = 512, 384
    key = (S_out, B_burn)
    if key not in _BUILT:
        _BUILT[key] = build(S_out=S_out, B_burn=B_burn)
    nc = _BUILT[key]
    in_maps = make_inputs_per_core(
        np.asarray(hidden_frames), np.asarray(Wc), np.asarray(bc),
        np.asarray(Wwg), np.asarray(bwg), np.asarray(Wwp), np.asarray(bwp),
        np.asarray(Wrg), np.asarray(brg), np.asarray(Wrp), np.asarray(brp),
        np.asarray(Wxh), np.asarray(Wrh), np.asarray(Whh), np.asarray(bh),
        S_out=S_out, B_burn=B_burn)
    res = bass_utils.run_bass_kernel_spmd(nc, in_maps, core_ids=list(range(NC)))
    return unpack_results(res, S_out=S_out)
